# revision 1
# baseline (speedup 1.0000x reference)
"""Trainium2 Bass kernel for nn_AttentionLayer_78632261255284 (sparse_attention).

Strategy (8-way row sharding, fully transpose-free):
  Each core owns a slab of 512 query rows. The reachability-factor matrix
  slab is computed *transposed* ([4096 keys, 512 queries]) via the chain
  D_{k+1} = B^T @ D_k with lhsT = B tiles, which is exactly the layout the
  transposed attention scores need. Softmax uses the identity
  softmax(qk + log f) = f*exp(qk) / sum(f*exp(qk)) -- no log, no max
  subtraction (scores are bounded). The softmax denominator comes from an
  appended ones-column in V; the division is applied per head via a rank-1
  (ones x recip-row) PE broadcast. The output projection consumes the
  transposed per-head outputs directly as lhsT. Host adds bo at the end.

Numerics: fp16 operand storage everywhere (validated: L2 rel err ~5e-4),
fp32 PSUM accumulation. Factors are globally scaled by 2^-9 (cancels in
softmax normalization exactly); D3 is stored scaled by 1/4 to stay in fp16
range. All scale constants are powers of two (exact).
"""

import numpy as np

import concourse.bass as bass
import concourse.mybir as mybir
import concourse.tile as tile
from concourse import bacc
from concourse.bass_utils import run_bass_kernel_spmd

P = 128
N = 4096  # nodes (+virtual)
NB = N // P  # 32 node blocks
EMB = 512
ET = EMB // P  # 4 embed blocks
HEADS = 8
HD = 64
SLAB = 512  # rows per core
NCORES = 8
CHUNK = 32  # kb-blocks per attention chunk (full-head sexp, double-buffered)

dt = mybir.dt
AF = mybir.ActivationFunctionType
ALU = mybir.AluOpType

# factor-scale constants (powers of two; global 2^-9 scale cancels in softmax)
FSCALE = 1.0 / 512.0
C2 = 0.5 * FSCALE
C3 = 0.25 * FSCALE
C4 = 0.125 * FSCALE
D3_STORE = 0.25  # D3 stored as D3/4 (fp16 range); level-3 psum is D4/4

_NC_CACHE = {}
LAST_RESULT = None


def _install_ntff_shim():
    """Provide antenv.axon_hooks if the image lacks it, so trace=True under
    axon works (profiling via ctypes into libaxon_pjrt.so). No-op if the
    real module exists or the .so lacks the symbols."""
    try:
        from antenv.axon_hooks import get_axon_ntff_profile_hook  # noqa: F401
        return
    except ImportError:
        pass
    import contextlib
    import ctypes
    import sys
    import types

    so_path = "/opt/axon/libaxon_pjrt.so"
    hook = None
    try:
        lib = ctypes.CDLL(so_path)
        if hasattr(lib, "axon_start_nrt_profile"):
            lib.axon_start_nrt_profile.argtypes = [
                ctypes.POINTER(ctypes.c_int64),
                ctypes.c_size_t,
            ]
            lib.axon_start_nrt_profile.restype = ctypes.c_int64
            lib.axon_stop_nrt_profile.argtypes = [ctypes.c_char_p]
            lib.axon_stop_nrt_profile.restype = ctypes.c_int64

            @contextlib.contextmanager
            def _hook(output_dir, device_ids):
                import jax

                jax.devices()
                if device_ids:
                    ids = (ctypes.c_int64 * len(device_ids))(*device_ids)
                    rc = lib.axon_start_nrt_profile(ids, len(device_ids))
                else:
                    rc = lib.axon_start_nrt_profile(None, 0)
                if rc != 0:
                    raise RuntimeError(f"axon_start_nrt_profile rc={rc}")
                try:
                    yield
                finally:
                    n = lib.axon_stop_nrt_profile(str(output_dir).encode())
                    if n < 0:
                        raise RuntimeError(f"axon_stop_nrt_profile rc={n}")

            hook = _hook
    except OSError:
        pass

    mod = types.ModuleType("antenv.axon_hooks")
    mod.get_axon_ntff_profile_hook = lambda: hook
    mod.set_axon_ntff_profile_hook = lambda h: None
    sys.modules["antenv.axon_hooks"] = mod


_install_ntff_shim()


def build_bass():
    nc = bacc.Bacc("TRN2", target_bir_lowering=False, debug=False, num_devices=NCORES)

    bt = nc.dram_tensor("bt", [NB, P, NB, P], dt.float16, kind="ExternalInput")
    bt8 = nc.dram_tensor("bt8", [NB, P, NB, P], dt.float8e4, kind="ExternalInput")
    d18 = nc.dram_tensor("d18", [N, SLAB], dt.float8e4, kind="ExternalInput")
    d1 = nc.dram_tensor("d1", [N, SLAB], dt.float16, kind="ExternalInput")
    xt = nc.dram_tensor("xt", [EMB, N], dt.float16, kind="ExternalInput")
    xtr = nc.dram_tensor("xtr", [EMB, SLAB], dt.float16, kind="ExternalInput")
    wq = nc.dram_tensor("wq", [EMB, EMB], dt.float16, kind="ExternalInput")
    wk = nc.dram_tensor("wk", [EMB, EMB], dt.float16, kind="ExternalInput")
    wv = nc.dram_tensor("wv", [EMB, EMB], dt.float16, kind="ExternalInput")
    wo = nc.dram_tensor("wo", [EMB, EMB], dt.float16, kind="ExternalInput")
    bq = nc.dram_tensor("bq", [EMB], dt.float32, kind="ExternalInput")
    bk = nc.dram_tensor("bk", [EMB], dt.float32, kind="ExternalInput")
    bv = nc.dram_tensor("bv", [EMB], dt.float32, kind="ExternalInput")
    out = nc.dram_tensor("out", [SLAB, EMB], dt.float32, kind="ExternalOutput")

    with tile.TileContext(nc) as tc:
        with (
            tc.tile_pool(name="persist", bufs=1) as persist,
            tc.tile_pool(name="psA", bufs=3, space="PSUM") as psA,
            tc.tile_pool(name="psB", bufs=3, space="PSUM") as psB,
            tc.tile_pool(name="psR", bufs=2, space="PSUM") as psR,
        ):
            # ---------------- persistent tiles ----------------
            F = persist.tile([P, NB, SLAB], dt.float16, tag="F")
            qT = persist.tile([P, ET, SLAB], dt.float16, tag="qT")
            out_allT = persist.tile([P, ET, SLAB], dt.float16, tag="out_allT")
            wo_sb = persist.tile([P, ET, EMB], dt.float16, tag="wo_sb")
            bv_sb = persist.tile([P, ET], dt.float32, tag="bv_sb")
            ones64 = persist.tile([1, HD], dt.float16, tag="ones64")

            nc.sync.dma_start(wo_sb[:], wo.rearrange("(t p) c -> p t c", p=P))
            nc.sync.dma_start(bv_sb[:], bv.rearrange("(t p) -> p t", p=P))
            nc.vector.memset(ones64[:], 1.0)

            # ---------------- phase M: mask chain ----------------
            with tc.tile_pool(name="dchain", bufs=1) as dchain, tc.tile_pool(
                name="btile", bufs=3
            ) as btile:
                D_a8 = dchain.tile([P, NB, SLAB], dt.float8e4, tag="D_a8")
                D_b = dchain.tile([P, NB, SLAB], dt.float16, tag="D_b")
                D_c = dchain.tile([P, NB, SLAB], dt.float16, tag="D_c")
                d18r = d18.rearrange("(kb p) q -> p kb q", p=P)
                for kb in range(NB):
                    nc.sync.dma_start(D_a8[:, kb, :], d18r[:, kb, :])

                # level 1: fp8e4m3 + DoubleRow (B and D1 entries are exact 0/1)
                for m in range(NB):
                    bm8 = btile.tile([P, NB, P], dt.float8e4, tag="bm8")
                    nc.sync.dma_start(bm8[:], bt8[m])
                    ps = psA.tile([P, SLAB], dt.float32, tag="acc")
                    for k2 in range(NB // 2):
                        nc.tensor.matmul(
                            ps[:],
                            bm8[:, 2 * k2 : 2 * k2 + 2, :],
                            D_a8[:, 2 * k2 : 2 * k2 + 2, :],
                            start=(k2 == 0),
                            stop=(k2 == NB // 2 - 1),
                            perf_mode=mybir.MatmulPerfMode.DoubleRow,
                        )
                    nc.vector.tensor_scalar_mul(F[:, m, :], ps[:], C2)
                    nc.scalar.copy(D_b[:, m, :], ps[:])

                # levels 2, 3: fp16
                for level in (1, 2):
                    src = D_b if level == 1 else D_c
                    dst = D_c if level == 1 else None
                    for m in range(NB):
                        bm = btile.tile([P, NB, P], dt.float16, tag="bm")
                        nc.sync.dma_start(bm[:], bt[m])
                        ps = psA.tile([P, SLAB], dt.float32, tag="acc")
                        for kb in range(NB):
                            nc.tensor.matmul(
                                ps[:],
                                bm[:, kb, :],
                                src[:, kb, :],
                                start=(kb == 0),
                                stop=(kb == NB - 1),
                            )
                        if level == 1:
                            nc.vector.scalar_tensor_tensor(
                                out=F[:, m, :], in0=ps[:], scalar=C3, in1=F[:, m, :],
                                op0=ALU.mult, op1=ALU.max,
                            )
                            nc.scalar.mul(dst[:, m, :], ps[:], D3_STORE)
                        else:
                            nc.vector.scalar_tensor_tensor(
                                out=F[:, m, :], in0=ps[:], scalar=C4 * 4.0, in1=F[:, m, :],
                                op0=ALU.mult, op1=ALU.max,
                            )

            # ---------------- phase P: projections ----------------
            with tc.tile_pool(name="kv", bufs=1) as kv:
                kT = kv.tile([P, ET, N], dt.float16, tag="kT")
                v_sb = kv.tile([P, NB, HEADS, HD + 1], dt.float16, tag="v_sb")

                with tc.tile_pool(name="proj", bufs=1) as proj:
                    xt_sb = proj.tile([P, ET, N], dt.float16, tag="xt_sb")
                    xtr_sb = proj.tile([P, ET, SLAB], dt.float16, tag="xtr_sb")
                    wq_sb = proj.tile([P, ET, EMB], dt.float16, tag="wq_sb")
                    wk_sb = proj.tile([P, ET, EMB], dt.float16, tag="wk_sb")
                    wv_sb = proj.tile([P, ET, EMB], dt.float16, tag="wv_sb")
                    bq_sb = proj.tile([P, ET], dt.float32, tag="bq_sb")
                    bk_sb = proj.tile([P, ET], dt.float32, tag="bk_sb")

                    nc.sync.dma_start(xt_sb[:], xt.rearrange("(t p) n -> p t n", p=P))
                    nc.sync.dma_start(xtr_sb[:], xtr.rearrange("(t p) q -> p t q", p=P))
                    nc.sync.dma_start(wq_sb[:], wq.rearrange("(t p) c -> p t c", p=P))
                    nc.sync.dma_start(wk_sb[:], wk.rearrange("(t p) c -> p t c", p=P))
                    nc.sync.dma_start(wv_sb[:], wv.rearrange("(t p) c -> p t c", p=P))
                    nc.sync.dma_start(bq_sb[:], bq.rearrange("(t p) -> p t", p=P))
                    nc.sync.dma_start(bk_sb[:], bk.rearrange("(t p) -> p t", p=P))

                    # qT[hd, q] = (Wq' X_r^T) + bq'
                    for hb in range(ET):
                        ps = psA.tile([P, SLAB], dt.float32, tag="acc")
                        for t in range(ET):
                            nc.tensor.matmul(
                                ps[:],
                                wq_sb[:, t, hb * P : (hb + 1) * P],
                                xtr_sb[:, t, :],
                                start=(t == 0),
                                stop=(t == ET - 1),
                            )
                        nc.scalar.activation(
                            qT[:, hb, :], ps[:], AF.Identity, bias=bq_sb[:, hb : hb + 1]
                        )

                    # kT[hd, key] = (Wk X^T) + bk
                    for hb in range(ET):
                        for nck in range(N // SLAB):
                            ps = psA.tile([P, SLAB], dt.float32, tag="acc")
                            for t in range(ET):
                                nc.tensor.matmul(
                                    ps[:],
                                    wk_sb[:, t, hb * P : (hb + 1) * P],
                                    xt_sb[:, t, nck * SLAB : (nck + 1) * SLAB],
                                    start=(t == 0),
                                    stop=(t == ET - 1),
                                )
                            nc.scalar.activation(
                                kT[:, hb, nck * SLAB : (nck + 1) * SLAB],
                                ps[:],
                                AF.Identity,
                                bias=bk_sb[:, hb : hb + 1],
                            )

                    # V[node, hd] = X Wv   (bv added later per-partition on out'^T)
                    for nb in range(NB):
                        ps = psA.tile([P, SLAB], dt.float32, tag="acc")
                        for t in range(ET):
                            nc.tensor.matmul(
                                ps[:],
                                xt_sb[:, t, nb * P : (nb + 1) * P],
                                wv_sb[:, t, :],
                                start=(t == 0),
                                stop=(t == ET - 1),
                            )
                        nc.vector.tensor_copy(
                            v_sb[:, nb, :, 0:HD],
                            ps.rearrange("p (h d) -> p h d", h=HEADS),
                        )
                    nc.vector.memset(v_sb[:, :, :, HD : HD + 1], 1.0)

                # ---------------- phase A: attention ----------------
                with tc.tile_pool(name="attn", bufs=2) as attn, tc.tile_pool(
                    name="small", bufs=3
                ) as small:
                    for h in range(HEADS):
                        th = h // 2
                        po = (h % 2) * HD
                        po_tile = psB.tile([P, SLAB], dt.float32, tag="pout")
                        for ck in range(NB // CHUNK):
                            sexp = attn.tile([P, CHUNK, SLAB], dt.float16, tag="sexp")
                            for kc in range(CHUNK):
                                kb = ck * CHUNK + kc
                                ps = psA.tile([P, SLAB], dt.float32, tag="acc")
                                nc.tensor.matmul(
                                    ps[:],
                                    kT[po : po + HD, th, kb * P : (kb + 1) * P],
                                    qT[po : po + HD, th, :],
                                    start=True,
                                    stop=True,
                                )
                                nc.scalar.activation(sexp[:, kc, :], ps[:], AF.Exp)
                                nc.vector.tensor_tensor(
                                    out=sexp[:, kc, :], in0=sexp[:, kc, :],
                                    in1=F[:, kb, :], op=ALU.mult,
                                )
                            for kc in range(CHUNK):
                                kb = ck * CHUNK + kc
                                nc.tensor.matmul(
                                    po_tile[0 : HD + 1, :],
                                    v_sb[:, kb, h, :],
                                    sexp[:, kc, :],
                                    start=(kb == 0),
                                    stop=(kb == NB - 1),
                                )

                        # softmax denominator: row HD holds sum(f*exp)
                        row = small.tile([1, SLAB], dt.float32, tag="row")
                        rscratch = small.tile([1, SLAB], dt.float32, tag="rscratch")
                        nc.vector.tensor_copy(row[:], po_tile[HD : HD + 1, :])
                        nc.vector.reciprocal_approx_accurate(
                            row[:], row[:], rscratch[:]
                        )
                        row16 = small.tile([1, SLAB], dt.float16, tag="row16")
                        nc.vector.tensor_copy(row16[:], row[:])
                        rps = psR.tile([HD, SLAB], dt.float32, tag="rps")
                        nc.tensor.matmul(rps[:], ones64[:], row16[:], start=True, stop=True)
                        r_sb = small.tile([HD, SLAB], dt.float32, tag="r_sb")
                        nc.scalar.copy(r_sb[:], rps[:])

                        otmp = small.tile([HD, SLAB], dt.float32, tag="otmp")
                        nc.vector.tensor_tensor(
                            out=otmp[:], in0=po_tile[0:HD, :], in1=r_sb[:], op=ALU.mult
                        )
                        nc.vector.tensor_scalar_add(
                            out_allT[po : po + HD, th, :], otmp[:],
                            bv_sb[po : po + HD, th : th + 1],
                        )

                    # ---------------- phase O: output projection ----------------
                    for qb in range(ET):
                        ps = psA.tile([P, SLAB], dt.float32, tag="acc")
                        for t in range(ET):
                            nc.tensor.matmul(
                                ps[:],
                                out_allT[:, t, qb * P : (qb + 1) * P],
                                wo_sb[:, t, :],
                                start=(t == 0),
                                stop=(t == ET - 1),
                            )
                        fin = small.tile([P, SLAB], dt.float32, tag="fin")
                        nc.vector.tensor_copy(fin[:], ps[:])
                        nc.sync.dma_start(out[qb * P : (qb + 1) * P, :], fin[:])

    nc.compile()
    return nc


def _prep_host(input_embeddings, edge_index, num_nodes, Wq, bq, Wk, bk, Wv, bv, Wo, bo):
    n = int(num_nodes) + 1
    assert n == N

    B = np.zeros((n, n), dtype=np.float32)
    idx = np.arange(n)
    B[idx, idx] = 1.0
    e0 = np.asarray(edge_index[0], dtype=np.int64)
    e1 = np.asarray(edge_index[1], dtype=np.int64)
    B[e0, e1] = 1.0
    B[: n - 1, n - 1] = 1.0
    B[n - 1, : n - 1] = 1.0

    B16 = B.astype(np.float16)
    fp8 = mybir.dt.np(dt.float8e4)
    # bt[m, p, kb, f] = B[kb*128+p, m*128+f]
    bt = np.ascontiguousarray(
        B16.reshape(NB, P, NB, P).transpose(2, 1, 0, 3)
    )
    bt8 = bt.astype(fp8)

    X = np.asarray(input_embeddings, dtype=np.float32)
    xt = np.ascontiguousarray(X.T.astype(np.float16))

    wq_h = np.ascontiguousarray((np.asarray(Wq, np.float32) * 0.125).astype(np.float16))
    wk_h = np.ascontiguousarray(np.asarray(Wk, np.float32).astype(np.float16))
    wv_h = np.ascontiguousarray(np.asarray(Wv, np.float32).astype(np.float16))
    wo_h = np.ascontiguousarray(np.asarray(Wo, np.float32).astype(np.float16))
    bq_h = np.ascontiguousarray(np.asarray(bq, np.float32) * 0.125)
    bk_h = np.ascontiguousarray(np.asarray(bk, np.float32))
    bv_h = np.ascontiguousarray(np.asarray(bv, np.float32))

    in_maps = []
    for core in range(NCORES):
        r0 = core * SLAB
        d1 = np.ascontiguousarray(B16[r0 : r0 + SLAB, :].T)
        d18_a = d1.astype(fp8)
        xtr = np.ascontiguousarray(xt[:, r0 : r0 + SLAB])
        in_maps.append(
            {
                "bt": bt,
                "bt8": bt8,
                "d18": d18_a,
                "d1": d1,
                "xt": xt,
                "xtr": xtr,
                "wq": wq_h,
                "wk": wk_h,
                "wv": wv_h,
                "wo": wo_h,
                "bq": bq_h,
                "bk": bk_h,
                "bv": bv_h,
            }
        )
    return in_maps


def kernel(**inputs) -> np.ndarray:
    if "nc" not in _NC_CACHE:
        _NC_CACHE["nc"] = build_bass()
    nc = _NC_CACHE["nc"]

    in_maps = _prep_host(**inputs)
    res = run_bass_kernel_spmd(nc, in_maps, core_ids=list(range(NCORES)))
    global LAST_RESULT
    LAST_RESULT = res
    bo = np.asarray(inputs["bo"], dtype=np.float32)
    slabs = [res.results[c]["out"] for c in range(NCORES)]
    return (np.concatenate(slabs, axis=0) + bo[None, :]).astype(np.float32)


if __name__ == "__main__":
    import reference

    inputs = {k: np.asarray(v) if not np.isscalar(v) else v for k, v in reference.setup_inputs().items()}
    got = kernel(**inputs)
    print("kernel output:", got.shape, got.dtype)



# revision 4
# speedup vs baseline: 1.3275x; 1.3275x over previous
"""Trainium2 Bass kernel for nn_AttentionLayer_78632261255284 (sparse_attention).

Strategy (8-way row sharding, fully transpose-free):
  Each core owns a slab of 512 query rows. The reachability-factor matrix
  slab is computed *transposed* ([4096 keys, 512 queries]) via the chain
  D_{k+1} = B^T @ D_k with lhsT = B tiles. All three chain levels run in
  fp8e4m3 with DoubleRow perf mode (2x PE throughput). The fp8 rounding of
  the D2/D3 operands is dominated by the virtual-node row (whose huge
  entries appear in every downstream sum via B's all-ones virtual row), so
  each level adds a rank-1 ones x residual correction matmul that restores
  the virtual row to ~fp16 accuracy (validated: rel err 2.8e-3 vs 2e-2
  gate). Softmax uses softmax(qk + log f) = f*exp(qk) / sum(f*exp(qk));
  the denominator comes from an appended ones-column in V. QK score
  matmuls (contract dim 64) run as row-tiled head pairs via tile_position
  (0,0)/(64,0), concurrent on the PE array, writing a 2-bank PSUM pair
  consumed by a single fused Exp activation (halves ACT PSUM-access
  overhead). Output projection consumes transposed per-head outputs as
  lhsT. Host adds bo at the end.

Numerics: fp16 operand storage on the value path, fp8e4m3 (max 240) for
the count-valued mask chain with power-of-2 scales: D2*2^-5, D3*2^-10,
residuals *2^5. All scale constants are powers of two (exact).
"""

import numpy as np

import concourse.bass as bass
import concourse.mybir as mybir
import concourse.tile as tile
from concourse import bacc
from concourse.bass_utils import run_bass_kernel_spmd

P = 128
N = 4096  # nodes (+virtual)
NB = N // P  # 32 node blocks
EMB = 512
ET = EMB // P  # 4 embed blocks
HEADS = 8
HD = 64
SLAB = 512  # rows per core
NCORES = 8
CHUNK = 16  # kb-blocks per attention chunk (per head pair, double-buffered)

dt = mybir.dt
AF = mybir.ActivationFunctionType
ALU = mybir.AluOpType

# factor-scale constants (powers of two; global 2^-9 scale cancels in softmax)
FSCALE = 1.0 / 512.0
C2 = 0.5 * FSCALE            # applied to L1 psum (raw D2)
C3 = 0.25 * FSCALE * 32.0    # applied to L2 psum (D3 * 2^-5)
C4 = 0.125 * FSCALE * 1024.0  # applied to L3 psum (D4 * 2^-10)
S2 = 1.0 / 32.0   # D2 fp8 storage scale
S3 = 1.0 / 32.0   # D3 fp8 storage scale relative to L2 psum (total 2^-10)
RS = 1.0 / 32.0   # residual rank-1 lhsT constant (undoes 2^5 residual upscale)

_NC_CACHE = {}
LAST_RESULT = None


def _install_ntff_shim():
    """Provide antenv.axon_hooks if the image lacks it, so trace=True under
    axon works (profiling via ctypes into libaxon_pjrt.so). No-op if the
    real module exists or the .so lacks the symbols."""
    try:
        from antenv.axon_hooks import get_axon_ntff_profile_hook  # noqa: F401
        return
    except ImportError:
        pass
    import contextlib
    import ctypes
    import sys
    import types

    so_path = "/opt/axon/libaxon_pjrt.so"
    hook = None
    try:
        lib = ctypes.CDLL(so_path)
        if hasattr(lib, "axon_start_nrt_profile"):
            lib.axon_start_nrt_profile.argtypes = [
                ctypes.POINTER(ctypes.c_int64),
                ctypes.c_size_t,
            ]
            lib.axon_start_nrt_profile.restype = ctypes.c_int64
            lib.axon_stop_nrt_profile.argtypes = [ctypes.c_char_p]
            lib.axon_stop_nrt_profile.restype = ctypes.c_int64

            @contextlib.contextmanager
            def _hook(output_dir, device_ids):
                import jax

                jax.devices()
                if device_ids:
                    ids = (ctypes.c_int64 * len(device_ids))(*device_ids)
                    rc = lib.axon_start_nrt_profile(ids, len(device_ids))
                else:
                    rc = lib.axon_start_nrt_profile(None, 0)
                if rc != 0:
                    raise RuntimeError(f"axon_start_nrt_profile rc={rc}")
                try:
                    yield
                finally:
                    n = lib.axon_stop_nrt_profile(str(output_dir).encode())
                    if n < 0:
                        raise RuntimeError(f"axon_stop_nrt_profile rc={n}")

            hook = _hook
    except OSError:
        pass

    mod = types.ModuleType("antenv.axon_hooks")
    mod.get_axon_ntff_profile_hook = lambda: hook
    mod.set_axon_ntff_profile_hook = lambda h: None
    sys.modules["antenv.axon_hooks"] = mod


_install_ntff_shim()


def build_bass():
    nc = bacc.Bacc("TRN2", target_bir_lowering=False, debug=False, num_devices=NCORES)

    bt8 = nc.dram_tensor("bt8", [NB, P, NB, P], dt.float8e4, kind="ExternalInput")
    d18 = nc.dram_tensor("d18", [N, SLAB], dt.float8e4, kind="ExternalInput")
    xt = nc.dram_tensor("xt", [EMB, N], dt.float16, kind="ExternalInput")
    xtr = nc.dram_tensor("xtr", [EMB, SLAB], dt.float16, kind="ExternalInput")
    wq = nc.dram_tensor("wq", [EMB, EMB], dt.float16, kind="ExternalInput")
    wk = nc.dram_tensor("wk", [EMB, EMB], dt.float16, kind="ExternalInput")
    wv = nc.dram_tensor("wv", [EMB, EMB], dt.float16, kind="ExternalInput")
    wo = nc.dram_tensor("wo", [EMB, EMB], dt.float16, kind="ExternalInput")
    bq = nc.dram_tensor("bq", [EMB], dt.float32, kind="ExternalInput")
    bk = nc.dram_tensor("bk", [EMB], dt.float32, kind="ExternalInput")
    bv = nc.dram_tensor("bv", [EMB], dt.float32, kind="ExternalInput")
    out = nc.dram_tensor("out", [SLAB, EMB], dt.float32, kind="ExternalOutput")

    # m-block order: virtual-node block first so the residual rows are
    # ready before the next level's rank-1 corrections need them.
    M_ORDER = [NB - 1] + list(range(NB - 1))

    with tile.TileContext(nc) as tc:
        with tc.tile_pool(name="persist", bufs=1) as persist:
            # ---------------- persistent tiles ----------------
            F = persist.tile([P, NB, SLAB], dt.float16, tag="F")
            qT = persist.tile([P, ET, SLAB], dt.float16, tag="qT")
            out_allT = persist.tile([P, ET, SLAB], dt.float16, tag="out_allT")
            wo_sb = persist.tile([P, ET, EMB], dt.float16, tag="wo_sb")
            bv_sb = persist.tile([P, ET], dt.float32, tag="bv_sb")
            ones64 = persist.tile([1, HD], dt.float16, tag="ones64")
            c32 = persist.tile([1, P], dt.float8e4, tag="c32")
            e_p0 = persist.tile([1, 2, SLAB], dt.float8e4, tag="e_p0")

            nc.sync.dma_start(wo_sb[:], wo.rearrange("(t p) c -> p t c", p=P))
            nc.sync.dma_start(bv_sb[:], bv.rearrange("(t p) -> p t", p=P))
            nc.vector.memset(ones64[:], 1.0)
            nc.vector.memset(c32[:], RS)

            # ---------------- phase M: mask chain (all fp8 DoubleRow) ----
            with (
                tc.tile_pool(name="dchain", bufs=1) as dchain,
                tc.tile_pool(name="btile", bufs=3) as btile,
                tc.tile_pool(name="psA", bufs=3, space="PSUM") as psA,
            ):
                D_a8 = dchain.tile([P, NB, SLAB], dt.float8e4, tag="D_a8")
                D2_8 = dchain.tile([P, NB, SLAB], dt.float8e4, tag="D2_8")
                D3_8 = dchain.tile([P, NB, SLAB], dt.float8e4, tag="D3_8")
                e_raw = dchain.tile([P, 2, SLAB], dt.float8e4, tag="e_raw")
                d18r = d18.rearrange("(kb p) q -> p kb q", p=P)
                for kb in range(NB):
                    nc.sync.dma_start(D_a8[:, kb, :], d18r[:, kb, :])

                # level 1: D2 = B^T D1 (exact 0/1 operands)
                for m in M_ORDER:
                    bm8 = btile.tile([P, NB, P], dt.float8e4, tag="bm8")
                    nc.sync.dma_start(bm8[:], bt8[m])
                    ps = psA.tile([P, SLAB], dt.float32, tag="acc")
                    for k2 in range(NB // 2):
                        nc.tensor.matmul(
                            ps[:],
                            bm8[:, 2 * k2 : 2 * k2 + 2, :],
                            D_a8[:, 2 * k2 : 2 * k2 + 2, :],
                            start=(k2 == 0),
                            stop=(k2 == NB // 2 - 1),
                            perf_mode=mybir.MatmulPerfMode.DoubleRow,
                        )
                    nc.vector.tensor_scalar_mul(F[:, m, :], ps[:], C2)
                    nc.scalar.mul(D2_8[:, m, :], ps[:], S2)
                    if m == NB - 1:
                        # E2*2^5 = ps[virt] - 32*D2_8[virt]  (residual; only
                        # row 127 = virtual node is used, computed full-block
                        # because DVE requires partition base 0)
                        nc.vector.scalar_tensor_tensor(
                            out=e_raw[:, 0, :],
                            in0=D2_8[:, NB - 1, :],
                            scalar=-32.0,
                            in1=ps[:],
                            op0=ALU.mult,
                            op1=ALU.add,
                        )
                        nc.sync.dma_start(e_p0[0:1, 0, :], e_raw[P - 1 : P, 0, :])

                # levels 2, 3: fp8 DR + rank-1 virtual-row residual correction
                for level in (2, 3):
                    src = D2_8 if level == 2 else D3_8
                    for m in M_ORDER:
                        bm8 = btile.tile([P, NB, P], dt.float8e4, tag="bm8")
                        nc.sync.dma_start(bm8[:], bt8[m])
                        ps = psA.tile([P, SLAB], dt.float32, tag="acc")
                        for k2 in range(NB // 2):
                            nc.tensor.matmul(
                                ps[:],
                                bm8[:, 2 * k2 : 2 * k2 + 2, :],
                                src[:, 2 * k2 : 2 * k2 + 2, :],
                                start=(k2 == 0),
                                stop=False,
                                perf_mode=mybir.MatmulPerfMode.DoubleRow,
                            )
                        nc.tensor.matmul(
                            ps[:],
                            c32[:],
                            e_p0[0:1, level - 2, :],
                            start=False,
                            stop=True,
                        )
                        nc.vector.scalar_tensor_tensor(
                            out=F[:, m, :], in0=ps[:],
                            scalar=(C3 if level == 2 else C4), in1=F[:, m, :],
                            op0=ALU.mult, op1=ALU.max,
                        )
                        if level == 2:
                            nc.scalar.mul(D3_8[:, m, :], ps[:], S3)
                            if m == NB - 1:
                                nc.vector.scalar_tensor_tensor(
                                    out=e_raw[:, 1, :],
                                    in0=D3_8[:, NB - 1, :],
                                    scalar=-32.0,
                                    in1=ps[:],
                                    op0=ALU.mult,
                                    op1=ALU.add,
                                )
                                nc.sync.dma_start(
                                    e_p0[0:1, 1, :], e_raw[P - 1 : P, 1, :]
                                )

            # ---------------- phase P: projections ----------------
            with tc.tile_pool(name="kv", bufs=1) as kv:
                kT = kv.tile([P, ET, N], dt.float16, tag="kT")
                v_sb = kv.tile([P, NB, HEADS, HD + 1], dt.float16, tag="v_sb")

                with (
                    tc.tile_pool(name="proj", bufs=1) as proj,
                    tc.tile_pool(name="psP", bufs=3, space="PSUM") as psP,
                ):
                    xt_sb = proj.tile([P, ET, N], dt.float16, tag="xt_sb")
                    xtr_sb = proj.tile([P, ET, SLAB], dt.float16, tag="xtr_sb")
                    wq_sb = proj.tile([P, ET, EMB], dt.float16, tag="wq_sb")
                    wk_sb = proj.tile([P, ET, EMB], dt.float16, tag="wk_sb")
                    wv_sb = proj.tile([P, ET, EMB], dt.float16, tag="wv_sb")
                    bq_sb = proj.tile([P, ET], dt.float32, tag="bq_sb")
                    bk_sb = proj.tile([P, ET], dt.float32, tag="bk_sb")

                    nc.sync.dma_start(xt_sb[:], xt.rearrange("(t p) n -> p t n", p=P))
                    nc.sync.dma_start(xtr_sb[:], xtr.rearrange("(t p) q -> p t q", p=P))
                    nc.sync.dma_start(wq_sb[:], wq.rearrange("(t p) c -> p t c", p=P))
                    nc.sync.dma_start(wk_sb[:], wk.rearrange("(t p) c -> p t c", p=P))
                    nc.sync.dma_start(wv_sb[:], wv.rearrange("(t p) c -> p t c", p=P))
                    nc.sync.dma_start(bq_sb[:], bq.rearrange("(t p) -> p t", p=P))
                    nc.sync.dma_start(bk_sb[:], bk.rearrange("(t p) -> p t", p=P))

                    # qT[hd, q] = (Wq' X_r^T) + bq'
                    for hb in range(ET):
                        ps = psP.tile([P, SLAB], dt.float32, tag="acc")
                        for t in range(ET):
                            nc.tensor.matmul(
                                ps[:],
                                wq_sb[:, t, hb * P : (hb + 1) * P],
                                xtr_sb[:, t, :],
                                start=(t == 0),
                                stop=(t == ET - 1),
                            )
                        nc.scalar.activation(
                            qT[:, hb, :], ps[:], AF.Identity, bias=bq_sb[:, hb : hb + 1]
                        )

                    # kT[hd, key] = (Wk X^T) + bk
                    for hb in range(ET):
                        for nck in range(N // SLAB):
                            ps = psP.tile([P, SLAB], dt.float32, tag="acc")
                            for t in range(ET):
                                nc.tensor.matmul(
                                    ps[:],
                                    wk_sb[:, t, hb * P : (hb + 1) * P],
                                    xt_sb[:, t, nck * SLAB : (nck + 1) * SLAB],
                                    start=(t == 0),
                                    stop=(t == ET - 1),
                                )
                            nc.scalar.activation(
                                kT[:, hb, nck * SLAB : (nck + 1) * SLAB],
                                ps[:],
                                AF.Identity,
                                bias=bk_sb[:, hb : hb + 1],
                            )

                    # V[node, hd] = X Wv   (bv added later per-partition on out'^T)
                    for nb in range(NB):
                        ps = psP.tile([P, SLAB], dt.float32, tag="acc")
                        for t in range(ET):
                            nc.tensor.matmul(
                                ps[:],
                                xt_sb[:, t, nb * P : (nb + 1) * P],
                                wv_sb[:, t, :],
                                start=(t == 0),
                                stop=(t == ET - 1),
                            )
                        nc.vector.tensor_copy(
                            v_sb[:, nb, :, 0:HD],
                            ps.rearrange("p (h d) -> p h d", h=HEADS),
                        )
                    nc.vector.memset(v_sb[:, :, :, HD : HD + 1], 1.0)

                # ---------------- phase A: attention (row-tiled head pairs) --
                with (
                    tc.tile_pool(name="attn", bufs=2) as attn,
                    tc.tile_pool(name="small", bufs=3) as small,
                    tc.tile_pool(name="psQK", bufs=2, space="PSUM") as psQK,
                    tc.tile_pool(name="psAV", bufs=1, space="PSUM") as psAV,
                    tc.tile_pool(name="psR", bufs=2, space="PSUM") as psR,
                ):
                    for t in range(HEADS // 2):
                        po_pair = psAV.tile([P, 2, SLAB], dt.float32, tag="pout")
                        for ck in range(NB // CHUNK):
                            sexp = attn.tile(
                                [P, CHUNK, 2, SLAB], dt.float16, tag="sexp"
                            )
                            for kc in range(CHUNK):
                                kb = ck * CHUNK + kc
                                ps = psQK.tile([P, 2, SLAB], dt.float32, tag="qk")
                                nc.tensor.matmul(
                                    ps[:, 0, :],
                                    kT[0:HD, t, kb * P : (kb + 1) * P],
                                    qT[0:HD, t, :],
                                    start=True,
                                    stop=True,
                                    tile_position=(0, 0),
                                )
                                nc.tensor.matmul(
                                    ps[:, 1, :],
                                    kT[HD:P, t, kb * P : (kb + 1) * P],
                                    qT[HD:P, t, :],
                                    start=True,
                                    stop=True,
                                    tile_position=(HD, 0),
                                )
                                nc.scalar.activation(sexp[:, kc, :, :], ps[:], AF.Exp)
                                nc.vector.tensor_tensor(
                                    out=sexp[:, kc, 0, :], in0=sexp[:, kc, 0, :],
                                    in1=F[:, kb, :], op=ALU.mult,
                                )
                                nc.vector.tensor_tensor(
                                    out=sexp[:, kc, 1, :], in0=sexp[:, kc, 1, :],
                                    in1=F[:, kb, :], op=ALU.mult,
                                )
                            for kc in range(CHUNK):
                                kb = ck * CHUNK + kc
                                for l in range(2):
                                    nc.tensor.matmul(
                                        po_pair[0 : HD + 1, l, :],
                                        v_sb[:, kb, 2 * t + l, :],
                                        sexp[:, kc, l, :],
                                        start=(kb == 0),
                                        stop=(kb == NB - 1),
                                    )

                        for l in range(2):
                            h = 2 * t + l
                            th = h // 2
                            po = (h % 2) * HD
                            # softmax denominator: row HD holds sum(f*exp)
                            row = small.tile([1, SLAB], dt.float32, tag="row")
                            rscratch = small.tile([1, SLAB], dt.float32, tag="rscratch")
                            nc.vector.tensor_copy(
                                row[:], po_pair[HD : HD + 1, l, :]
                            )
                            nc.vector.reciprocal_approx_accurate(
                                row[:], row[:], rscratch[:]
                            )
                            row16 = small.tile([1, SLAB], dt.float16, tag="row16")
                            nc.vector.tensor_copy(row16[:], row[:])
                            rps = psR.tile([HD, SLAB], dt.float32, tag="rps")
                            nc.tensor.matmul(
                                rps[:], ones64[:], row16[:], start=True, stop=True
                            )
                            r_sb = small.tile([HD, SLAB], dt.float32, tag="r_sb")
                            nc.scalar.copy(r_sb[:], rps[:])

                            otmp = small.tile([HD, SLAB], dt.float32, tag="otmp")
                            nc.vector.tensor_tensor(
                                out=otmp[:], in0=po_pair[0:HD, l, :], in1=r_sb[:],
                                op=ALU.mult,
                            )
                            nc.vector.tensor_scalar_add(
                                out_allT[po : po + HD, th, :], otmp[:],
                                bv_sb[po : po + HD, th : th + 1],
                            )

                # ---------------- phase O: output projection ----------------
                with (
                    tc.tile_pool(name="osmall", bufs=2) as osmall,
                    tc.tile_pool(name="psO", bufs=2, space="PSUM") as psO,
                ):
                    for qb in range(ET):
                        ps = psO.tile([P, SLAB], dt.float32, tag="acc")
                        for t in range(ET):
                            nc.tensor.matmul(
                                ps[:],
                                out_allT[:, t, qb * P : (qb + 1) * P],
                                wo_sb[:, t, :],
                                start=(t == 0),
                                stop=(t == ET - 1),
                            )
                        fin = osmall.tile([P, SLAB], dt.float32, tag="fin")
                        nc.vector.tensor_copy(fin[:], ps[:])
                        nc.sync.dma_start(out[qb * P : (qb + 1) * P, :], fin[:])

    nc.compile()
    return nc


def _prep_host(input_embeddings, edge_index, num_nodes, Wq, bq, Wk, bk, Wv, bv, Wo, bo):
    n = int(num_nodes) + 1
    assert n == N

    B = np.zeros((n, n), dtype=np.float32)
    idx = np.arange(n)
    B[idx, idx] = 1.0
    e0 = np.asarray(edge_index[0], dtype=np.int64)
    e1 = np.asarray(edge_index[1], dtype=np.int64)
    B[e0, e1] = 1.0
    B[: n - 1, n - 1] = 1.0
    B[n - 1, : n - 1] = 1.0

    fp8 = mybir.dt.np(dt.float8e4)
    # bt8[m, p, kb, f] = B[kb*128+p, m*128+f]
    bt8 = np.ascontiguousarray(
        B.reshape(NB, P, NB, P).transpose(2, 1, 0, 3)
    ).astype(fp8)

    X = np.asarray(input_embeddings, dtype=np.float32)
    xt = np.ascontiguousarray(X.T.astype(np.float16))

    wq_h = np.ascontiguousarray((np.asarray(Wq, np.float32) * 0.125).astype(np.float16))
    wk_h = np.ascontiguousarray(np.asarray(Wk, np.float32).astype(np.float16))
    wv_h = np.ascontiguousarray(np.asarray(Wv, np.float32).astype(np.float16))
    wo_h = np.ascontiguousarray(np.asarray(Wo, np.float32).astype(np.float16))
    bq_h = np.ascontiguousarray(np.asarray(bq, np.float32) * 0.125)
    bk_h = np.ascontiguousarray(np.asarray(bk, np.float32))
    bv_h = np.ascontiguousarray(np.asarray(bv, np.float32))

    in_maps = []
    for core in range(NCORES):
        r0 = core * SLAB
        d18_a = np.ascontiguousarray(B[r0 : r0 + SLAB, :].T).astype(fp8)
        xtr = np.ascontiguousarray(xt[:, r0 : r0 + SLAB])
        in_maps.append(
            {
                "bt8": bt8,
                "d18": d18_a,
                "xt": xt,
                "xtr": xtr,
                "wq": wq_h,
                "wk": wk_h,
                "wv": wv_h,
                "wo": wo_h,
                "bq": bq_h,
                "bk": bk_h,
                "bv": bv_h,
            }
        )
    return in_maps


def kernel(**inputs) -> np.ndarray:
    if "nc" not in _NC_CACHE:
        _NC_CACHE["nc"] = build_bass()
    nc = _NC_CACHE["nc"]

    in_maps = _prep_host(**inputs)
    res = run_bass_kernel_spmd(nc, in_maps, core_ids=list(range(NCORES)))
    global LAST_RESULT
    LAST_RESULT = res
    bo = np.asarray(inputs["bo"], dtype=np.float32)
    slabs = [res.results[c]["out"] for c in range(NCORES)]
    return (np.concatenate(slabs, axis=0) + bo[None, :]).astype(np.float32)


if __name__ == "__main__":
    import reference

    inputs = {k: np.asarray(v) if not np.isscalar(v) else v for k, v in reference.setup_inputs().items()}
    got = kernel(**inputs)
    print("kernel output:", got.shape, got.dtype)


# revision 12
# speedup vs baseline: 1.3491x; 1.0163x over previous
"""Trainium2 Bass kernel for nn_AttentionLayer_78632261255284 (sparse_attention).

Strategy (8-way row sharding, fully transpose-free):
  Each core owns a slab of 512 query rows. The reachability-factor matrix
  slab is computed *transposed* ([4096 keys, 512 queries]) via the chain
  D_{k+1} = B^T @ D_k with lhsT = B tiles. All three chain levels run in
  fp8e4m3 with DoubleRow perf mode (2x PE throughput). The fp8 rounding of
  the D2/D3 operands is dominated by the virtual-node row (whose huge
  entries appear in every downstream sum via B's all-ones virtual row), so
  each level adds a rank-1 ones x residual correction matmul that restores
  the virtual row to ~fp16 accuracy (validated: rel err 2.8e-3 vs 2e-2
  gate). Softmax uses softmax(qk + log f) = f*exp(qk) / sum(f*exp(qk));
  the denominator comes from an appended ones-column in V. QK score
  matmuls (contract dim 64) run as row-tiled head pairs via tile_position
  (0,0)/(64,0), concurrent on the PE array, writing a 2-bank PSUM pair
  consumed by a single fused Exp activation (halves ACT PSUM-access
  overhead). Output projection consumes transposed per-head outputs as
  lhsT. Host adds bo at the end.

Numerics: fp16 operand storage on the value path, fp8e4m3 (max 240) for
the count-valued mask chain with power-of-2 scales: D2*2^-5, D3*2^-10,
residuals *2^5. All scale constants are powers of two (exact).
"""

import numpy as np

import concourse.bass as bass
import concourse.mybir as mybir
import concourse.tile as tile
from concourse import bacc
from concourse.bass_utils import run_bass_kernel_spmd

P = 128
N = 4096  # nodes (+virtual)
NB = N // P  # 32 node blocks
EMB = 512
ET = EMB // P  # 4 embed blocks
HEADS = 8
HD = 64
SLAB = 512  # rows per core
NCORES = 8
CHUNK = 16  # kb-blocks per attention chunk (per head pair, double-buffered)

dt = mybir.dt
AF = mybir.ActivationFunctionType
ALU = mybir.AluOpType

# factor-scale constants (powers of two; global 2^-9 scale cancels in softmax)
FSCALE = 1.0 / 512.0
C2 = 0.5 * FSCALE            # applied to L1 psum (raw D2)
C3 = 0.25 * FSCALE * 32.0    # applied to L2 psum (D3 * 2^-5)
C4 = 0.125 * FSCALE * 1024.0  # applied to L3 psum (D4 * 2^-10)
S2 = 1.0 / 32.0   # D2 fp8 storage scale
S3 = 1.0 / 32.0   # D3 fp8 storage scale relative to L2 psum (total 2^-10)
RS = 1.0 / 32.0   # residual rank-1 lhsT constant (undoes 2^5 residual upscale)

_NC_CACHE = {}
LAST_RESULT = None


def _install_ntff_shim():
    """Provide antenv.axon_hooks if the image lacks it, so trace=True under
    axon works (profiling via ctypes into libaxon_pjrt.so). No-op if the
    real module exists or the .so lacks the symbols."""
    try:
        from antenv.axon_hooks import get_axon_ntff_profile_hook  # noqa: F401
        return
    except ImportError:
        pass
    import contextlib
    import ctypes
    import sys
    import types

    so_path = "/opt/axon/libaxon_pjrt.so"
    hook = None
    try:
        lib = ctypes.CDLL(so_path)
        if hasattr(lib, "axon_start_nrt_profile"):
            lib.axon_start_nrt_profile.argtypes = [
                ctypes.POINTER(ctypes.c_int64),
                ctypes.c_size_t,
            ]
            lib.axon_start_nrt_profile.restype = ctypes.c_int64
            lib.axon_stop_nrt_profile.argtypes = [ctypes.c_char_p]
            lib.axon_stop_nrt_profile.restype = ctypes.c_int64

            @contextlib.contextmanager
            def _hook(output_dir, device_ids):
                import jax

                jax.devices()
                if device_ids:
                    ids = (ctypes.c_int64 * len(device_ids))(*device_ids)
                    rc = lib.axon_start_nrt_profile(ids, len(device_ids))
                else:
                    rc = lib.axon_start_nrt_profile(None, 0)
                if rc != 0:
                    raise RuntimeError(f"axon_start_nrt_profile rc={rc}")
                try:
                    yield
                finally:
                    n = lib.axon_stop_nrt_profile(str(output_dir).encode())
                    if n < 0:
                        raise RuntimeError(f"axon_stop_nrt_profile rc={n}")

            hook = _hook
    except OSError:
        pass

    mod = types.ModuleType("antenv.axon_hooks")
    mod.get_axon_ntff_profile_hook = lambda: hook
    mod.set_axon_ntff_profile_hook = lambda h: None
    sys.modules["antenv.axon_hooks"] = mod


_install_ntff_shim()


def build_bass():
    nc = bacc.Bacc("TRN2", target_bir_lowering=False, debug=False, num_devices=NCORES)

    bt8 = nc.dram_tensor("bt8", [NB, P, NB, P], dt.float8e4, kind="ExternalInput")
    d18 = nc.dram_tensor("d18", [N, SLAB], dt.float8e4, kind="ExternalInput")
    xt = nc.dram_tensor("xt", [EMB, N], dt.float16, kind="ExternalInput")
    xtr = nc.dram_tensor("xtr", [EMB, SLAB], dt.float16, kind="ExternalInput")
    wq = nc.dram_tensor("wq", [EMB, EMB], dt.float16, kind="ExternalInput")
    wk = nc.dram_tensor("wk", [EMB, EMB], dt.float16, kind="ExternalInput")
    wv = nc.dram_tensor("wv", [EMB, EMB], dt.float16, kind="ExternalInput")
    wo = nc.dram_tensor("wo", [EMB, EMB], dt.float16, kind="ExternalInput")
    bq = nc.dram_tensor("bq", [EMB], dt.float32, kind="ExternalInput")
    bk = nc.dram_tensor("bk", [EMB], dt.float32, kind="ExternalInput")
    bv = nc.dram_tensor("bv", [EMB], dt.float32, kind="ExternalInput")
    out = nc.dram_tensor("out", [SLAB, EMB], dt.float32, kind="ExternalOutput")

    # m-block order: virtual-node block first so the residual rows are
    # ready before the next level's rank-1 corrections need them.
    M_ORDER = [NB - 1] + list(range(NB - 1))

    with tile.TileContext(nc) as tc:
        with tc.tile_pool(name="persist", bufs=1) as persist:
            # ---------------- persistent tiles ----------------
            F = persist.tile([P, NB, SLAB], dt.float16, tag="F")
            qT = persist.tile([P, ET, SLAB], dt.float16, tag="qT")
            out_allT = persist.tile([P, ET, SLAB], dt.float16, tag="out_allT")
            wo_sb = persist.tile([P, ET, EMB], dt.float16, tag="wo_sb")
            bv_sb = persist.tile([P, ET], dt.float32, tag="bv_sb")
            c32 = persist.tile([1, P], dt.float8e4, tag="c32")
            e_p0 = persist.tile([1, 2, SLAB], dt.float8e4, tag="e_p0")
            ones64 = persist.tile([1, HD], dt.float16, tag="ones64")

            nc.vector.memset(c32[:], RS)
            nc.vector.memset(ones64[:], 1.0)

            # ---------------- phase M: mask chain (all fp8 DoubleRow) ----
            with (
                tc.tile_pool(name="dchain", bufs=1) as dchain,
                tc.tile_pool(name="btile", bufs=3) as btile,
                tc.tile_pool(name="psA", bufs=3, space="PSUM") as psA,
            ):
                D_a8 = dchain.tile([P, NB, SLAB], dt.float8e4, tag="D_a8")
                D2_8 = dchain.tile([P, NB, SLAB], dt.float8e4, tag="D2_8")
                D3_8 = dchain.tile([P, NB, SLAB], dt.float8e4, tag="D3_8")
                e_raw = dchain.tile([P, 2, SLAB], dt.float8e4, tag="e_raw")
                nc.sync.dma_start(D_a8[:], d18.rearrange("(kb p) q -> p kb q", p=P))
                # weight loads for later phases, behind the mask-critical DMAs
                nc.sync.dma_start(wo_sb[:], wo.rearrange("(t p) c -> p t c", p=P))
                nc.sync.dma_start(bv_sb[:], bv.rearrange("(t p) -> p t", p=P))

                # level 1: D2 = B^T D1 (exact 0/1 operands)
                for m in M_ORDER:
                    bm8 = btile.tile([P, NB, P], dt.float8e4, tag="bm8")
                    nc.sync.dma_start(bm8[:], bt8[m])
                    ps = psA.tile([P, SLAB], dt.float32, tag="acc")
                    for k2 in range(NB // 2):
                        nc.tensor.matmul(
                            ps[:],
                            bm8[:, 2 * k2 : 2 * k2 + 2, :],
                            D_a8[:, 2 * k2 : 2 * k2 + 2, :],
                            start=(k2 == 0),
                            stop=(k2 == NB // 2 - 1),
                            perf_mode=mybir.MatmulPerfMode.DoubleRow,
                        )
                    nc.scalar.mul(D2_8[:, m, :], ps[:], S2)
                    if m == NB - 1:
                        # E2*2^5 = ps[virt] - 32*D2_8[virt]  (residual; only
                        # row 127 = virtual node is used, computed full-block
                        # because DVE requires partition base 0)
                        nc.vector.scalar_tensor_tensor(
                            out=e_raw[:, 0, :],
                            in0=D2_8[:, NB - 1, :],
                            scalar=-32.0,
                            in1=ps[:],
                            op0=ALU.mult,
                            op1=ALU.add,
                        )
                        nc.sync.dma_start(e_p0[0:1, 0, :], e_raw[P - 1 : P, 0, :])

                # levels 2, 3: fp8 DR + rank-1 virtual-row residual correction
                for level in (2, 3):
                    src = D2_8 if level == 2 else D3_8
                    for m in M_ORDER:
                        bm8 = btile.tile([P, NB, P], dt.float8e4, tag="bm8")
                        nc.sync.dma_start(bm8[:], bt8[m])
                        ps = psA.tile([P, SLAB], dt.float32, tag="acc")
                        for k2 in range(NB // 2):
                            nc.tensor.matmul(
                                ps[:],
                                bm8[:, 2 * k2 : 2 * k2 + 2, :],
                                src[:, 2 * k2 : 2 * k2 + 2, :],
                                start=(k2 == 0),
                                stop=False,
                                perf_mode=mybir.MatmulPerfMode.DoubleRow,
                            )
                        nc.tensor.matmul(
                            ps[:],
                            c32[:],
                            e_p0[0:1, level - 2, :],
                            start=False,
                            stop=True,
                        )
                        # F = 0.125*B^4 exactly: the virtual node's all-ones
                        # row/col makes the B^2/B^3/eye max-terms redundant
                        # (B^4 >= 2*B^3 and >= 4*B^2 entrywise, proven + verified)
                        if level == 3:
                            nc.vector.tensor_scalar_mul(F[:, m, :], ps[:], C4)
                        if level == 2:
                            nc.scalar.mul(D3_8[:, m, :], ps[:], S3)
                            if m == NB - 1:
                                nc.vector.scalar_tensor_tensor(
                                    out=e_raw[:, 1, :],
                                    in0=D3_8[:, NB - 1, :],
                                    scalar=-32.0,
                                    in1=ps[:],
                                    op0=ALU.mult,
                                    op1=ALU.add,
                                )
                                nc.sync.dma_start(
                                    e_p0[0:1, 1, :], e_raw[P - 1 : P, 1, :]
                                )

            # ---------------- phase P: projections ----------------
            with tc.tile_pool(name="kv", bufs=1) as kv:
                kT = kv.tile([P, ET, N], dt.float16, tag="kT")
                v_sb = kv.tile([P, NB, HEADS, HD + 1], dt.float16, tag="v_sb")

                with (
                    tc.tile_pool(name="proj", bufs=1) as proj,
                    tc.tile_pool(name="psP", bufs=3, space="PSUM") as psP,
                ):
                    xt_sb = proj.tile([P, ET, N], dt.float16, tag="xt_sb")
                    xtr_sb = proj.tile([P, ET, SLAB], dt.float16, tag="xtr_sb")
                    wq_sb = proj.tile([P, ET, EMB], dt.float16, tag="wq_sb")
                    wk_sb = proj.tile([P, ET, EMB], dt.float16, tag="wk_sb")
                    wv_sb = proj.tile([P, ET, EMB], dt.float16, tag="wv_sb")
                    bq_sb = proj.tile([P, ET], dt.float32, tag="bq_sb")
                    bk_sb = proj.tile([P, ET], dt.float32, tag="bk_sb")

                    nc.sync.dma_start(xt_sb[:], xt.rearrange("(t p) n -> p t n", p=P))
                    nc.sync.dma_start(xtr_sb[:], xtr.rearrange("(t p) q -> p t q", p=P))
                    nc.sync.dma_start(wq_sb[:], wq.rearrange("(t p) c -> p t c", p=P))
                    nc.sync.dma_start(wk_sb[:], wk.rearrange("(t p) c -> p t c", p=P))
                    nc.sync.dma_start(wv_sb[:], wv.rearrange("(t p) c -> p t c", p=P))
                    nc.sync.dma_start(bq_sb[:], bq.rearrange("(t p) -> p t", p=P))
                    nc.sync.dma_start(bk_sb[:], bk.rearrange("(t p) -> p t", p=P))

                    # qT[hd, q] = (Wq' X_r^T) + bq'
                    for hb in range(ET):
                        ps = psP.tile([P, SLAB], dt.float32, tag="acc")
                        for t in range(ET):
                            nc.tensor.matmul(
                                ps[:],
                                wq_sb[:, t, hb * P : (hb + 1) * P],
                                xtr_sb[:, t, :],
                                start=(t == 0),
                                stop=(t == ET - 1),
                            )
                        nc.scalar.activation(
                            qT[:, hb, :], ps[:], AF.Identity, bias=bq_sb[:, hb : hb + 1]
                        )

                    # kT[hd, key] = (Wk X^T) + bk
                    for hb in range(ET):
                        for nck in range(N // SLAB):
                            ps = psP.tile([P, SLAB], dt.float32, tag="acc")
                            for t in range(ET):
                                nc.tensor.matmul(
                                    ps[:],
                                    wk_sb[:, t, hb * P : (hb + 1) * P],
                                    xt_sb[:, t, nck * SLAB : (nck + 1) * SLAB],
                                    start=(t == 0),
                                    stop=(t == ET - 1),
                                )
                            nc.scalar.activation(
                                kT[:, hb, nck * SLAB : (nck + 1) * SLAB],
                                ps[:],
                                AF.Identity,
                                bias=bk_sb[:, hb : hb + 1],
                            )

                    # V[node, hd] = X Wv   (bv added later per-partition on out'^T)
                    for nb in range(NB):
                        ps = psP.tile([P, SLAB], dt.float32, tag="acc")
                        for t in range(ET):
                            nc.tensor.matmul(
                                ps[:],
                                xt_sb[:, t, nb * P : (nb + 1) * P],
                                wv_sb[:, t, :],
                                start=(t == 0),
                                stop=(t == ET - 1),
                            )
                        nc.vector.tensor_copy(
                            v_sb[:, nb, :, 0:HD],
                            ps.rearrange("p (h d) -> p h d", h=HEADS),
                        )
                    nc.vector.memset(v_sb[:, :, :, HD : HD + 1], 1.0)

                # ---------------- phase A: attention (row-tiled head pairs) --
                with (
                    tc.tile_pool(name="attn", bufs=2) as attn,
                    tc.tile_pool(name="small", bufs=3) as small,
                    tc.tile_pool(name="psQK", bufs=3, space="PSUM") as psQK,
                    tc.tile_pool(name="psAV", bufs=1, space="PSUM") as psAV,
                ):
                    for t in range(HEADS // 2):
                        po_pair = psAV.tile([P, 2, SLAB], dt.float32, tag="pout")
                        for ck in range(NB // CHUNK):
                            sexp = attn.tile(
                                [P, CHUNK, 2, SLAB], dt.float16, tag="sexp"
                            )
                            for kc in range(CHUNK):
                                kb = ck * CHUNK + kc
                                ps = psQK.tile([P, 2, SLAB], dt.float32, tag="qk")
                                nc.tensor.matmul(
                                    ps[:, 0, :],
                                    kT[0:HD, t, kb * P : (kb + 1) * P],
                                    qT[0:HD, t, :],
                                    start=True,
                                    stop=True,
                                    tile_position=(0, 0),
                                )
                                nc.tensor.matmul(
                                    ps[:, 1, :],
                                    kT[HD:P, t, kb * P : (kb + 1) * P],
                                    qT[HD:P, t, :],
                                    start=True,
                                    stop=True,
                                    tile_position=(HD, 0),
                                )
                                nc.scalar.activation(sexp[:, kc, :, :], ps[:], AF.Exp)
                                nc.vector.tensor_tensor(
                                    out=sexp[:, kc, 0, :], in0=sexp[:, kc, 0, :],
                                    in1=F[:, kb, :], op=ALU.mult,
                                )
                                nc.vector.tensor_tensor(
                                    out=sexp[:, kc, 1, :], in0=sexp[:, kc, 1, :],
                                    in1=F[:, kb, :], op=ALU.mult,
                                )
                            for kc in range(CHUNK):
                                kb = ck * CHUNK + kc
                                for l in range(2):
                                    nc.tensor.matmul(
                                        po_pair[0 : HD + 1, l, :],
                                        v_sb[:, kb, 2 * t + l, :],
                                        sexp[:, kc, l, :],
                                        start=(kb == 0),
                                        stop=(kb == NB - 1),
                                    )

                        for l in range(2):
                            h = 2 * t + l
                            th = h // 2
                            po = (h % 2) * HD
                            # softmax denominator: row HD holds sum(f*exp)
                            row = small.tile([1, SLAB], dt.float32, tag="row")
                            rscratch = small.tile([1, SLAB], dt.float32, tag="rscratch")
                            nc.vector.tensor_copy(
                                row[:], po_pair[HD : HD + 1, l, :]
                            )
                            nc.vector.reciprocal_approx_accurate(
                                row[:], row[:], rscratch[:]
                            )
                            row16 = small.tile([1, SLAB], dt.float16, tag="row16")
                            nc.vector.tensor_copy(row16[:], row[:])
                            rps = psQK.tile([P, 2, SLAB], dt.float32, tag="qk")
                            nc.tensor.matmul(
                                rps[0:HD, 0, :], ones64[:], row16[:],
                                start=True, stop=True,
                            )
                            r_sb = small.tile([HD, SLAB], dt.float32, tag="r_sb")
                            nc.scalar.copy(r_sb[:], rps[0:HD, 0, :])

                            otmp = small.tile([HD, SLAB], dt.float32, tag="otmp")
                            nc.vector.tensor_tensor(
                                out=otmp[:], in0=po_pair[0:HD, l, :],
                                in1=r_sb[:], op=ALU.mult,
                            )
                            nc.vector.tensor_scalar_add(
                                out_allT[po : po + HD, th, :], otmp[:],
                                bv_sb[po : po + HD, th : th + 1],
                            )

                # ---------------- phase O: output projection ----------------
                with (
                    tc.tile_pool(name="osmall", bufs=2) as osmall,
                    tc.tile_pool(name="psO", bufs=2, space="PSUM") as psO,
                ):
                    for qb in range(ET):
                        ps = psO.tile([P, SLAB], dt.float32, tag="acc")
                        for t in range(ET):
                            nc.tensor.matmul(
                                ps[:],
                                out_allT[:, t, qb * P : (qb + 1) * P],
                                wo_sb[:, t, :],
                                start=(t == 0),
                                stop=(t == ET - 1),
                            )
                        fin = osmall.tile([P, SLAB], dt.float32, tag="fin")
                        nc.vector.tensor_copy(fin[:], ps[:])
                        nc.sync.dma_start(out[qb * P : (qb + 1) * P, :], fin[:])

    nc.compile()
    return nc


def _prep_host(input_embeddings, edge_index, num_nodes, Wq, bq, Wk, bk, Wv, bv, Wo, bo):
    n = int(num_nodes) + 1
    assert n == N

    B = np.zeros((n, n), dtype=np.float32)
    idx = np.arange(n)
    B[idx, idx] = 1.0
    e0 = np.asarray(edge_index[0], dtype=np.int64)
    e1 = np.asarray(edge_index[1], dtype=np.int64)
    B[e0, e1] = 1.0
    B[: n - 1, n - 1] = 1.0
    B[n - 1, : n - 1] = 1.0

    fp8 = mybir.dt.np(dt.float8e4)
    # bt8[m, p, kb, f] = B[kb*128+p, m*128+f]
    bt8 = np.ascontiguousarray(
        B.reshape(NB, P, NB, P).transpose(2, 1, 0, 3)
    ).astype(fp8)

    X = np.asarray(input_embeddings, dtype=np.float32)
    xt = np.ascontiguousarray(X.T.astype(np.float16))

    wq_h = np.ascontiguousarray((np.asarray(Wq, np.float32) * 0.125).astype(np.float16))
    wk_h = np.ascontiguousarray(np.asarray(Wk, np.float32).astype(np.float16))
    wv_h = np.ascontiguousarray(np.asarray(Wv, np.float32).astype(np.float16))
    wo_h = np.ascontiguousarray(np.asarray(Wo, np.float32).astype(np.float16))
    bq_h = np.ascontiguousarray(np.asarray(bq, np.float32) * 0.125)
    bk_h = np.ascontiguousarray(np.asarray(bk, np.float32))
    bv_h = np.ascontiguousarray(np.asarray(bv, np.float32))

    in_maps = []
    for core in range(NCORES):
        r0 = core * SLAB
        d18_a = np.ascontiguousarray(B[r0 : r0 + SLAB, :].T).astype(fp8)
        xtr = np.ascontiguousarray(xt[:, r0 : r0 + SLAB])
        in_maps.append(
            {
                "bt8": bt8,
                "d18": d18_a,
                "xt": xt,
                "xtr": xtr,
                "wq": wq_h,
                "wk": wk_h,
                "wv": wv_h,
                "wo": wo_h,
                "bq": bq_h,
                "bk": bk_h,
                "bv": bv_h,
            }
        )
    return in_maps


def kernel(**inputs) -> np.ndarray:
    if "nc" not in _NC_CACHE:
        _NC_CACHE["nc"] = build_bass()
    nc = _NC_CACHE["nc"]

    in_maps = _prep_host(**inputs)
    res = run_bass_kernel_spmd(nc, in_maps, core_ids=list(range(NCORES)))
    global LAST_RESULT
    LAST_RESULT = res
    bo = np.asarray(inputs["bo"], dtype=np.float32)
    slabs = [res.results[c]["out"] for c in range(NCORES)]
    return (np.concatenate(slabs, axis=0) + bo[None, :]).astype(np.float32)


if __name__ == "__main__":
    import reference

    inputs = {k: np.asarray(v) if not np.isscalar(v) else v for k, v in reference.setup_inputs().items()}
    got = kernel(**inputs)
    print("kernel output:", got.shape, got.dtype)


# revision 15
# speedup vs baseline: 1.3536x; 1.0033x over previous
"""Trainium2 Bass kernel for nn_AttentionLayer_78632261255284 (sparse_attention).

Strategy (8-way row sharding, fully transpose-free):
  Each core owns a slab of 512 query rows. The reachability-factor matrix
  slab is computed *transposed* ([4096 keys, 512 queries]) via the chain
  D_{k+1} = B^T @ D_k with lhsT = B tiles. All three chain levels run in
  fp8e4m3 with DoubleRow perf mode (2x PE throughput). The fp8 rounding of
  the D2/D3 operands is dominated by the virtual-node row (whose huge
  entries appear in every downstream sum via B's all-ones virtual row), so
  each level adds a rank-1 ones x residual correction matmul that restores
  the virtual row to ~fp16 accuracy (validated: rel err 2.8e-3 vs 2e-2
  gate). Softmax uses softmax(qk + log f) = f*exp(qk) / sum(f*exp(qk));
  the denominator comes from an appended ones-column in V. QK score
  matmuls (contract dim 64) run as row-tiled head pairs via tile_position
  (0,0)/(64,0), concurrent on the PE array, writing a 2-bank PSUM pair
  consumed by a single fused Exp activation (halves ACT PSUM-access
  overhead). Output projection consumes transposed per-head outputs as
  lhsT. Host adds bo at the end.

Numerics: fp16 operand storage on the value path, fp8e4m3 (max 240) for
the count-valued mask chain with power-of-2 scales: D2*2^-5, D3*2^-10,
residuals *2^5. All scale constants are powers of two (exact).
"""

import numpy as np

import concourse.bass as bass
import concourse.mybir as mybir
import concourse.tile as tile
from concourse import bacc
from concourse.bass_utils import run_bass_kernel_spmd

P = 128
N = 4096  # nodes (+virtual)
NB = N // P  # 32 node blocks
EMB = 512
ET = EMB // P  # 4 embed blocks
HEADS = 8
HD = 64
SLAB = 512  # rows per core
NCORES = 8
CHUNK = 16  # kb-blocks per attention chunk (per head pair, double-buffered)

dt = mybir.dt
AF = mybir.ActivationFunctionType
ALU = mybir.AluOpType

# factor-scale constants (powers of two; global 2^-9 scale cancels in softmax)
FSCALE = 1.0 / 512.0
C2 = 0.5 * FSCALE            # applied to L1 psum (raw D2)
C3 = 0.25 * FSCALE * 32.0    # applied to L2 psum (D3 * 2^-5)
C4 = 0.125 * FSCALE * 1024.0  # applied to L3 psum (D4 * 2^-10)
S2 = 1.0 / 32.0   # D2 fp8 storage scale
S3 = 1.0 / 32.0   # D3 fp8 storage scale relative to L2 psum (total 2^-10)
RS = 1.0 / 32.0   # residual rank-1 lhsT constant (undoes 2^5 residual upscale)

_NC_CACHE = {}
LAST_RESULT = None


def _install_ntff_shim():
    """Provide antenv.axon_hooks if the image lacks it, so trace=True under
    axon works (profiling via ctypes into libaxon_pjrt.so). No-op if the
    real module exists or the .so lacks the symbols."""
    try:
        from antenv.axon_hooks import get_axon_ntff_profile_hook  # noqa: F401
        return
    except ImportError:
        pass
    import contextlib
    import ctypes
    import sys
    import types

    so_path = "/opt/axon/libaxon_pjrt.so"
    hook = None
    try:
        lib = ctypes.CDLL(so_path)
        if hasattr(lib, "axon_start_nrt_profile"):
            lib.axon_start_nrt_profile.argtypes = [
                ctypes.POINTER(ctypes.c_int64),
                ctypes.c_size_t,
            ]
            lib.axon_start_nrt_profile.restype = ctypes.c_int64
            lib.axon_stop_nrt_profile.argtypes = [ctypes.c_char_p]
            lib.axon_stop_nrt_profile.restype = ctypes.c_int64

            @contextlib.contextmanager
            def _hook(output_dir, device_ids):
                import jax

                jax.devices()
                if device_ids:
                    ids = (ctypes.c_int64 * len(device_ids))(*device_ids)
                    rc = lib.axon_start_nrt_profile(ids, len(device_ids))
                else:
                    rc = lib.axon_start_nrt_profile(None, 0)
                if rc != 0:
                    raise RuntimeError(f"axon_start_nrt_profile rc={rc}")
                try:
                    yield
                finally:
                    n = lib.axon_stop_nrt_profile(str(output_dir).encode())
                    if n < 0:
                        raise RuntimeError(f"axon_stop_nrt_profile rc={n}")

            hook = _hook
    except OSError:
        pass

    mod = types.ModuleType("antenv.axon_hooks")
    mod.get_axon_ntff_profile_hook = lambda: hook
    mod.set_axon_ntff_profile_hook = lambda h: None
    sys.modules["antenv.axon_hooks"] = mod


_install_ntff_shim()


def build_bass():
    nc = bacc.Bacc("TRN2", target_bir_lowering=False, debug=False, num_devices=NCORES)

    bt8 = nc.dram_tensor("bt8", [NB, P, NB, P], dt.float8e4, kind="ExternalInput")
    d18 = nc.dram_tensor("d18", [N, SLAB], dt.float8e4, kind="ExternalInput")
    xt = nc.dram_tensor("xt", [EMB, N], dt.float16, kind="ExternalInput")
    xtr = nc.dram_tensor("xtr", [EMB, SLAB], dt.float16, kind="ExternalInput")
    wq = nc.dram_tensor("wq", [EMB, EMB], dt.float16, kind="ExternalInput")
    wk = nc.dram_tensor("wk", [EMB, EMB], dt.float16, kind="ExternalInput")
    wv = nc.dram_tensor("wv", [EMB, EMB], dt.float16, kind="ExternalInput")
    wo = nc.dram_tensor("wo", [EMB, EMB], dt.float16, kind="ExternalInput")
    bq = nc.dram_tensor("bq", [EMB], dt.float32, kind="ExternalInput")
    bk = nc.dram_tensor("bk", [EMB], dt.float32, kind="ExternalInput")
    bv = nc.dram_tensor("bv", [EMB], dt.float32, kind="ExternalInput")
    out = nc.dram_tensor("out", [SLAB, EMB], dt.float32, kind="ExternalOutput")

    # m-block order: virtual-node block first so the residual rows are
    # ready before the next level's rank-1 corrections need them.
    M_ORDER = [NB - 1] + list(range(NB - 1))

    with tile.TileContext(nc) as tc:
        with tc.tile_pool(name="persist", bufs=1) as persist:
            # ---------------- persistent tiles ----------------
            F = persist.tile([P, NB, SLAB], dt.float16, tag="F")
            qT = persist.tile([P, ET, SLAB], dt.float16, tag="qT")
            out_allT = persist.tile([P, ET, SLAB], dt.float16, tag="out_allT")
            wo_sb = persist.tile([P, ET, EMB], dt.float16, tag="wo_sb")
            bv_sb = persist.tile([P, ET], dt.float32, tag="bv_sb")
            c32 = persist.tile([1, P], dt.float8e4, tag="c32")
            e_p0 = persist.tile([1, 2, SLAB], dt.float8e4, tag="e_p0")
            ones64 = persist.tile([1, HD], dt.float16, tag="ones64")

            nc.vector.memset(c32[:], RS)
            nc.vector.memset(ones64[:], 1.0)

            # ---------------- phase M: mask chain (all fp8 DoubleRow) ----
            with (
                tc.tile_pool(name="dchain", bufs=1) as dchain,
                tc.tile_pool(name="btile", bufs=3) as btile,
                tc.tile_pool(name="psA", bufs=3, space="PSUM") as psA,
            ):
                D_a8 = dchain.tile([P, NB, SLAB], dt.float8e4, tag="D_a8")
                D2_8 = dchain.tile([P, NB, SLAB], dt.float8e4, tag="D2_8")
                D3_8 = dchain.tile([P, NB, SLAB], dt.float8e4, tag="D3_8")
                e_raw = dchain.tile([P, 2, SLAB], dt.float8e4, tag="e_raw")
                e_tmp = dchain.tile([P, SLAB], dt.float16, tag="e_tmp")
                corr2_d = dchain.tile([P, SLAB], dt.float16, tag="corr2_d")
                corr2_raw = dchain.tile([P, SLAB], dt.float16, tag="corr2_raw")
                corr3F = dchain.tile([P, SLAB], dt.float16, tag="corr3F")
                nc.sync.dma_start(D_a8[:], d18.rearrange("(kb p) q -> p kb q", p=P))
                # weight loads for later phases, behind the mask-critical DMAs
                nc.sync.dma_start(wo_sb[:], wo.rearrange("(t p) c -> p t c", p=P))
                nc.sync.dma_start(bv_sb[:], bv.rearrange("(t p) -> p t", p=P))

                # level 1: D2 = B^T D1 (exact 0/1 operands)
                for m in M_ORDER:
                    bm8 = btile.tile([P, NB, P], dt.float8e4, tag="bm8")
                    nc.sync.dma_start(bm8[:], bt8[m])
                    ps = psA.tile([P, SLAB], dt.float32, tag="acc")
                    for k2 in range(NB // 2):
                        nc.tensor.matmul(
                            ps[:],
                            bm8[:, 2 * k2 : 2 * k2 + 2, :],
                            D_a8[:, 2 * k2 : 2 * k2 + 2, :],
                            start=(k2 == 0),
                            stop=(k2 == NB // 2 - 1),
                            perf_mode=mybir.MatmulPerfMode.DoubleRow,
                        )
                    nc.scalar.mul(D2_8[:, m, :], ps[:], S2)
                    if m == NB - 1:
                        # E2*2^5 = ps[virt] - 32*D2_8[virt]  (residual; only
                        # row 127 = virtual node is used, computed full-block
                        # because DVE requires partition base 0)
                        nc.vector.scalar_tensor_tensor(
                            out=e_raw[:, 0, :],
                            in0=D2_8[:, NB - 1, :],
                            scalar=-32.0,
                            in1=ps[:],
                            op0=ALU.mult,
                            op1=ALU.add,
                        )
                        nc.sync.dma_start(e_p0[0:1, 0, :], e_raw[P - 1 : P, 0, :])
                        # broadcast E2 across partitions once (replaces a
                        # rank-1 correction matmul in every level-2 m-block)
                        ps_c = psA.tile([P, SLAB], dt.float32, tag="acc")
                        nc.tensor.matmul(
                            ps_c[:], c32[:], e_p0[0:1, 0, :], start=True, stop=True
                        )
                        nc.scalar.mul(corr2_d[:], ps_c[:], S3)
                        nc.scalar.copy(corr2_raw[:], ps_c[:])

                # levels 2, 3: fp8 DR; the virtual-row fp8 residual enters via
                # broadcast correction tiles added in the psum consumers.
                # F = 0.125*B^4 exactly: the virtual node's all-ones row/col
                # makes the B^2/B^3/eye max-terms redundant (B^4 >= 2*B^3 and
                # >= 4*B^2 entrywise, proven + verified).
                for level in (2, 3):
                    src = D2_8 if level == 2 else D3_8
                    for m in M_ORDER:
                        bm8 = btile.tile([P, NB, P], dt.float8e4, tag="bm8")
                        nc.sync.dma_start(bm8[:], bt8[m])
                        ps = psA.tile([P, SLAB], dt.float32, tag="acc")
                        for k2 in range(NB // 2):
                            nc.tensor.matmul(
                                ps[:],
                                bm8[:, 2 * k2 : 2 * k2 + 2, :],
                                src[:, 2 * k2 : 2 * k2 + 2, :],
                                start=(k2 == 0),
                                stop=(k2 == NB // 2 - 1),
                                perf_mode=mybir.MatmulPerfMode.DoubleRow,
                            )
                        if level == 3:
                            # F = (ps + E3_bcast) * 0.25
                            nc.vector.scalar_tensor_tensor(
                                out=F[:, m, :], in0=ps[:], scalar=C4,
                                in1=corr3F[:], op0=ALU.mult, op1=ALU.add,
                            )
                        if level == 2:
                            # D3_8 = (ps + E2_bcast) * S3
                            nc.vector.scalar_tensor_tensor(
                                out=D3_8[:, m, :], in0=ps[:], scalar=S3,
                                in1=corr2_d[:], op0=ALU.mult, op1=ALU.add,
                            )
                            if m == NB - 1:
                                nc.vector.scalar_tensor_tensor(
                                    out=e_tmp[:],
                                    in0=D3_8[:, NB - 1, :],
                                    scalar=-32.0,
                                    in1=ps[:],
                                    op0=ALU.mult,
                                    op1=ALU.add,
                                )
                                nc.vector.tensor_tensor(
                                    out=e_raw[:, 1, :], in0=e_tmp[:],
                                    in1=corr2_raw[:], op=ALU.add,
                                )
                                nc.sync.dma_start(
                                    e_p0[0:1, 1, :], e_raw[P - 1 : P, 1, :]
                                )
                                ps_c = psA.tile([P, SLAB], dt.float32, tag="acc")
                                nc.tensor.matmul(
                                    ps_c[:], c32[:], e_p0[0:1, 1, :],
                                    start=True, stop=True,
                                )
                                nc.scalar.mul(corr3F[:], ps_c[:], C4)

            # ---------------- phase P: projections ----------------
            with tc.tile_pool(name="kv", bufs=1) as kv:
                kT = kv.tile([P, ET, N], dt.float16, tag="kT")
                v_sb = kv.tile([P, NB, HEADS, HD + 1], dt.float16, tag="v_sb")

                with (
                    tc.tile_pool(name="proj", bufs=1) as proj,
                    tc.tile_pool(name="psP", bufs=3, space="PSUM") as psP,
                ):
                    xt_sb = proj.tile([P, ET, N], dt.float16, tag="xt_sb")
                    xtr_sb = proj.tile([P, ET, SLAB], dt.float16, tag="xtr_sb")
                    wq_sb = proj.tile([P, ET, EMB], dt.float16, tag="wq_sb")
                    wk_sb = proj.tile([P, ET, EMB], dt.float16, tag="wk_sb")
                    wv_sb = proj.tile([P, ET, EMB], dt.float16, tag="wv_sb")
                    bq_sb = proj.tile([P, ET], dt.float32, tag="bq_sb")
                    bk_sb = proj.tile([P, ET], dt.float32, tag="bk_sb")

                    nc.sync.dma_start(xt_sb[:], xt.rearrange("(t p) n -> p t n", p=P))
                    nc.sync.dma_start(xtr_sb[:], xtr.rearrange("(t p) q -> p t q", p=P))
                    nc.sync.dma_start(wq_sb[:], wq.rearrange("(t p) c -> p t c", p=P))
                    nc.sync.dma_start(wk_sb[:], wk.rearrange("(t p) c -> p t c", p=P))
                    nc.sync.dma_start(wv_sb[:], wv.rearrange("(t p) c -> p t c", p=P))
                    nc.sync.dma_start(bq_sb[:], bq.rearrange("(t p) -> p t", p=P))
                    nc.sync.dma_start(bk_sb[:], bk.rearrange("(t p) -> p t", p=P))

                    # qT[hd, q] = (Wq' X_r^T) + bq'
                    for hb in range(ET):
                        ps = psP.tile([P, SLAB], dt.float32, tag="acc")
                        for t in range(ET):
                            nc.tensor.matmul(
                                ps[:],
                                wq_sb[:, t, hb * P : (hb + 1) * P],
                                xtr_sb[:, t, :],
                                start=(t == 0),
                                stop=(t == ET - 1),
                            )
                        nc.scalar.activation(
                            qT[:, hb, :], ps[:], AF.Identity, bias=bq_sb[:, hb : hb + 1]
                        )

                    # kT[hd, key] = (Wk X^T) + bk
                    for hb in range(ET):
                        for nck in range(N // SLAB):
                            ps = psP.tile([P, SLAB], dt.float32, tag="acc")
                            for t in range(ET):
                                nc.tensor.matmul(
                                    ps[:],
                                    wk_sb[:, t, hb * P : (hb + 1) * P],
                                    xt_sb[:, t, nck * SLAB : (nck + 1) * SLAB],
                                    start=(t == 0),
                                    stop=(t == ET - 1),
                                )
                            nc.scalar.activation(
                                kT[:, hb, nck * SLAB : (nck + 1) * SLAB],
                                ps[:],
                                AF.Identity,
                                bias=bk_sb[:, hb : hb + 1],
                            )

                    # V[node, hd] = X Wv   (bv added later per-partition on out'^T)
                    for nb in range(NB):
                        ps = psP.tile([P, SLAB], dt.float32, tag="acc")
                        for t in range(ET):
                            nc.tensor.matmul(
                                ps[:],
                                xt_sb[:, t, nb * P : (nb + 1) * P],
                                wv_sb[:, t, :],
                                start=(t == 0),
                                stop=(t == ET - 1),
                            )
                        nc.vector.tensor_copy(
                            v_sb[:, nb, :, 0:HD],
                            ps.rearrange("p (h d) -> p h d", h=HEADS),
                        )
                    nc.vector.memset(v_sb[:, :, :, HD : HD + 1], 1.0)

                # ---------------- phase A: attention (row-tiled head pairs) --
                with (
                    tc.tile_pool(name="attn", bufs=2) as attn,
                    tc.tile_pool(name="small", bufs=3) as small,
                    tc.tile_pool(name="psQK", bufs=3, space="PSUM") as psQK,
                    tc.tile_pool(name="psAV", bufs=1, space="PSUM") as psAV,
                ):
                    for t in range(HEADS // 2):
                        po_pair = psAV.tile([P, 2, SLAB], dt.float32, tag="pout")
                        for ck in range(NB // CHUNK):
                            sexp = attn.tile(
                                [P, CHUNK, 2, SLAB], dt.float16, tag="sexp"
                            )
                            for kc in range(CHUNK):
                                kb = ck * CHUNK + kc
                                ps = psQK.tile([P, 2, SLAB], dt.float32, tag="qk")
                                nc.tensor.matmul(
                                    ps[:, 0, :],
                                    kT[0:HD, t, kb * P : (kb + 1) * P],
                                    qT[0:HD, t, :],
                                    start=True,
                                    stop=True,
                                    tile_position=(0, 0),
                                )
                                nc.tensor.matmul(
                                    ps[:, 1, :],
                                    kT[HD:P, t, kb * P : (kb + 1) * P],
                                    qT[HD:P, t, :],
                                    start=True,
                                    stop=True,
                                    tile_position=(HD, 0),
                                )
                                nc.scalar.activation(sexp[:, kc, :, :], ps[:], AF.Exp)
                                nc.vector.tensor_tensor(
                                    out=sexp[:, kc, 0, :], in0=sexp[:, kc, 0, :],
                                    in1=F[:, kb, :], op=ALU.mult,
                                )
                                nc.vector.tensor_tensor(
                                    out=sexp[:, kc, 1, :], in0=sexp[:, kc, 1, :],
                                    in1=F[:, kb, :], op=ALU.mult,
                                )
                            for kc in range(CHUNK):
                                kb = ck * CHUNK + kc
                                for l in range(2):
                                    nc.tensor.matmul(
                                        po_pair[0 : HD + 1, l, :],
                                        v_sb[:, kb, 2 * t + l, :],
                                        sexp[:, kc, l, :],
                                        start=(kb == 0),
                                        stop=(kb == NB - 1),
                                    )

                        for l in range(2):
                            h = 2 * t + l
                            th = h // 2
                            po = (h % 2) * HD
                            # softmax denominator: row HD holds sum(f*exp)
                            row = small.tile([1, SLAB], dt.float32, tag="row")
                            rscratch = small.tile([1, SLAB], dt.float32, tag="rscratch")
                            nc.vector.tensor_copy(
                                row[:], po_pair[HD : HD + 1, l, :]
                            )
                            nc.vector.reciprocal_approx_accurate(
                                row[:], row[:], rscratch[:]
                            )
                            row16 = small.tile([1, SLAB], dt.float16, tag="row16")
                            nc.vector.tensor_copy(row16[:], row[:])
                            rps = psQK.tile([P, 2, SLAB], dt.float32, tag="qk")
                            nc.tensor.matmul(
                                rps[0:HD, 0, :], ones64[:], row16[:],
                                start=True, stop=True,
                            )
                            r_sb = small.tile([HD, SLAB], dt.float32, tag="r_sb")
                            nc.scalar.copy(r_sb[:], rps[0:HD, 0, :])

                            otmp = small.tile([HD, SLAB], dt.float32, tag="otmp")
                            nc.vector.tensor_tensor(
                                out=otmp[:], in0=po_pair[0:HD, l, :],
                                in1=r_sb[:], op=ALU.mult,
                            )
                            nc.vector.tensor_scalar_add(
                                out_allT[po : po + HD, th, :], otmp[:],
                                bv_sb[po : po + HD, th : th + 1],
                            )

                # ---------------- phase O: output projection ----------------
                with (
                    tc.tile_pool(name="osmall", bufs=2) as osmall,
                    tc.tile_pool(name="psO", bufs=2, space="PSUM") as psO,
                ):
                    for qb in range(ET):
                        ps = psO.tile([P, SLAB], dt.float32, tag="acc")
                        for t in range(ET):
                            nc.tensor.matmul(
                                ps[:],
                                out_allT[:, t, qb * P : (qb + 1) * P],
                                wo_sb[:, t, :],
                                start=(t == 0),
                                stop=(t == ET - 1),
                            )
                        fin = osmall.tile([P, SLAB], dt.float32, tag="fin")
                        nc.vector.tensor_copy(fin[:], ps[:])
                        nc.sync.dma_start(out[qb * P : (qb + 1) * P, :], fin[:])

    nc.compile()
    return nc


def _prep_host(input_embeddings, edge_index, num_nodes, Wq, bq, Wk, bk, Wv, bv, Wo, bo):
    n = int(num_nodes) + 1
    assert n == N

    B = np.zeros((n, n), dtype=np.float32)
    idx = np.arange(n)
    B[idx, idx] = 1.0
    e0 = np.asarray(edge_index[0], dtype=np.int64)
    e1 = np.asarray(edge_index[1], dtype=np.int64)
    B[e0, e1] = 1.0
    B[: n - 1, n - 1] = 1.0
    B[n - 1, : n - 1] = 1.0

    fp8 = mybir.dt.np(dt.float8e4)
    # bt8[m, p, kb, f] = B[kb*128+p, m*128+f]
    bt8 = np.ascontiguousarray(
        B.reshape(NB, P, NB, P).transpose(2, 1, 0, 3)
    ).astype(fp8)

    X = np.asarray(input_embeddings, dtype=np.float32)
    xt = np.ascontiguousarray(X.T.astype(np.float16))

    wq_h = np.ascontiguousarray((np.asarray(Wq, np.float32) * 0.125).astype(np.float16))
    wk_h = np.ascontiguousarray(np.asarray(Wk, np.float32).astype(np.float16))
    wv_h = np.ascontiguousarray(np.asarray(Wv, np.float32).astype(np.float16))
    wo_h = np.ascontiguousarray(np.asarray(Wo, np.float32).astype(np.float16))
    bq_h = np.ascontiguousarray(np.asarray(bq, np.float32) * 0.125)
    bk_h = np.ascontiguousarray(np.asarray(bk, np.float32))
    bv_h = np.ascontiguousarray(np.asarray(bv, np.float32))

    in_maps = []
    for core in range(NCORES):
        r0 = core * SLAB
        d18_a = np.ascontiguousarray(B[r0 : r0 + SLAB, :].T).astype(fp8)
        xtr = np.ascontiguousarray(xt[:, r0 : r0 + SLAB])
        in_maps.append(
            {
                "bt8": bt8,
                "d18": d18_a,
                "xt": xt,
                "xtr": xtr,
                "wq": wq_h,
                "wk": wk_h,
                "wv": wv_h,
                "wo": wo_h,
                "bq": bq_h,
                "bk": bk_h,
                "bv": bv_h,
            }
        )
    return in_maps


def kernel(**inputs) -> np.ndarray:
    if "nc" not in _NC_CACHE:
        _NC_CACHE["nc"] = build_bass()
    nc = _NC_CACHE["nc"]

    in_maps = _prep_host(**inputs)
    res = run_bass_kernel_spmd(nc, in_maps, core_ids=list(range(NCORES)))
    global LAST_RESULT
    LAST_RESULT = res
    bo = np.asarray(inputs["bo"], dtype=np.float32)
    slabs = [res.results[c]["out"] for c in range(NCORES)]
    return (np.concatenate(slabs, axis=0) + bo[None, :]).astype(np.float32)


if __name__ == "__main__":
    import reference

    inputs = {k: np.asarray(v) if not np.isscalar(v) else v for k, v in reference.setup_inputs().items()}
    got = kernel(**inputs)
    print("kernel output:", got.shape, got.dtype)


# revision 17
# speedup vs baseline: 1.3888x; 1.0261x over previous
"""Trainium2 Bass kernel for nn_AttentionLayer_78632261255284 (sparse_attention).

Strategy (8-way row sharding, fully transpose-free):
  Each core owns a slab of 512 query rows. The reachability-factor matrix
  slab is computed *transposed* ([4096 keys, 512 queries]) via the chain
  D_{k+1} = B^T @ D_k with lhsT = B tiles. All three chain levels run in
  fp8e4m3 with DoubleRow perf mode (2x PE throughput). F = 0.125*B^4
  EXACTLY: the virtual node's all-ones row/column makes the eye/B^2/B^3
  max-terms redundant (B^4 >= 2*B^3 >= ... entrywise; proven and verified
  numerically). The fp8 rounding of the D2/D3 operands is dominated by the
  virtual-node row (whose huge entries appear in every downstream sum via
  B's all-ones virtual row), so each level adds a broadcast residual
  correction tile (rank-1 built once, folded into the per-block DVE
  consumer ops) that restores the virtual row to ~fp16 accuracy
  (validated: rel err 2.8e-3 sim / 9.6e-4 hw vs 2e-2 gate).

  Softmax uses softmax(qk + log f) = f*exp(qk) / sum(f*exp(qk)); the
  denominator comes from an appended ones-column in V. QK score matmuls
  (contract dim 64) run as row-tiled head pairs via tile_position
  (0,0)/(64,0), concurrent on the PE array, writing a 2-bank PSUM pair
  consumed by a single fused Exp activation. Pair normalization is
  deferred past the next pair's first QK chunk so the PE never idles on
  the DVE reciprocal chain. Projection operand loads are issued on the
  Activation engine's DMA queue during the mask phase so the projection
  phase starts with data resident. Output projection consumes transposed
  per-head outputs as lhsT. Host adds bo at the end.

Numerics: fp16 operand storage on the value path, fp8e4m3 (max 240) for
the count-valued mask chain with power-of-2 scales: D2*2^-5, D3*2^-10,
residuals *2^5. All scale constants are powers of two (exact).
"""

import numpy as np

import concourse.bass as bass
import concourse.mybir as mybir
import concourse.tile as tile
from concourse import bacc
from concourse.bass_utils import run_bass_kernel_spmd

P = 128
N = 4096  # nodes (+virtual)
NB = N // P  # 32 node blocks
EMB = 512
ET = EMB // P  # 4 embed blocks
HEADS = 8
HD = 64
SLAB = 512  # rows per core
NCORES = 8
CHUNK = 8  # kb-blocks per attention chunk (per head pair, double-buffered)

dt = mybir.dt
AF = mybir.ActivationFunctionType
ALU = mybir.AluOpType

# factor-scale constants (powers of two; global 2^-9 scale cancels in softmax)
FSCALE = 1.0 / 512.0
C4 = 0.125 * FSCALE * 1024.0  # applied to L3 psum (D4 * 2^-10)
S2 = 1.0 / 32.0   # D2 fp8 storage scale
S3 = 1.0 / 32.0   # D3 fp8 storage scale relative to L2 psum (total 2^-10)
RS = 1.0 / 32.0   # residual broadcast lhsT constant (undoes 2^5 upscale)

_NC_CACHE = {}
LAST_RESULT = None


def _install_ntff_shim():
    """Provide antenv.axon_hooks if the image lacks it, so trace=True under
    axon works (profiling via ctypes into libaxon_pjrt.so). No-op if the
    real module exists or the .so lacks the symbols."""
    try:
        from antenv.axon_hooks import get_axon_ntff_profile_hook  # noqa: F401
        return
    except ImportError:
        pass
    import contextlib
    import ctypes
    import sys
    import types

    so_path = "/opt/axon/libaxon_pjrt.so"
    hook = None
    try:
        lib = ctypes.CDLL(so_path)
        if hasattr(lib, "axon_start_nrt_profile"):
            lib.axon_start_nrt_profile.argtypes = [
                ctypes.POINTER(ctypes.c_int64),
                ctypes.c_size_t,
            ]
            lib.axon_start_nrt_profile.restype = ctypes.c_int64
            lib.axon_stop_nrt_profile.argtypes = [ctypes.c_char_p]
            lib.axon_stop_nrt_profile.restype = ctypes.c_int64

            @contextlib.contextmanager
            def _hook(output_dir, device_ids):
                import jax

                jax.devices()
                if device_ids:
                    ids = (ctypes.c_int64 * len(device_ids))(*device_ids)
                    rc = lib.axon_start_nrt_profile(ids, len(device_ids))
                else:
                    rc = lib.axon_start_nrt_profile(None, 0)
                if rc != 0:
                    raise RuntimeError(f"axon_start_nrt_profile rc={rc}")
                try:
                    yield
                finally:
                    n = lib.axon_stop_nrt_profile(str(output_dir).encode())
                    if n < 0:
                        raise RuntimeError(f"axon_stop_nrt_profile rc={n}")

            hook = _hook
    except OSError:
        pass

    mod = types.ModuleType("antenv.axon_hooks")
    mod.get_axon_ntff_profile_hook = lambda: hook
    mod.set_axon_ntff_profile_hook = lambda h: None
    sys.modules["antenv.axon_hooks"] = mod


_install_ntff_shim()


def build_bass():
    nc = bacc.Bacc("TRN2", target_bir_lowering=False, debug=False, num_devices=NCORES)

    bt8 = nc.dram_tensor("bt8", [NB, P, NB, P], dt.float8e4, kind="ExternalInput")
    d18 = nc.dram_tensor("d18", [N, SLAB], dt.float8e4, kind="ExternalInput")
    xt = nc.dram_tensor("xt", [EMB, N], dt.float16, kind="ExternalInput")
    xtr = nc.dram_tensor("xtr", [EMB, SLAB], dt.float16, kind="ExternalInput")
    wq = nc.dram_tensor("wq", [EMB, EMB], dt.float16, kind="ExternalInput")
    wk = nc.dram_tensor("wk", [EMB, EMB], dt.float16, kind="ExternalInput")
    wv = nc.dram_tensor("wv", [EMB, EMB], dt.float16, kind="ExternalInput")
    wo = nc.dram_tensor("wo", [EMB, EMB], dt.float16, kind="ExternalInput")
    bq = nc.dram_tensor("bq", [EMB], dt.float32, kind="ExternalInput")
    bk = nc.dram_tensor("bk", [EMB], dt.float32, kind="ExternalInput")
    bv = nc.dram_tensor("bv", [EMB], dt.float32, kind="ExternalInput")
    out = nc.dram_tensor("out", [SLAB, EMB], dt.float32, kind="ExternalOutput")

    # m-block order: virtual-node block first so the residual rows are
    # ready before the next level's broadcast corrections need them.
    M_ORDER = [NB - 1] + list(range(NB - 1))

    with tile.TileContext(nc) as tc:
        with (
            tc.tile_pool(name="persist", bufs=1) as persist,
            tc.tile_pool(name="projd", bufs=1) as projd,
        ):
            # ---------------- persistent tiles ----------------
            F = persist.tile([P, NB, SLAB], dt.float16, tag="F")
            qT = persist.tile([P, ET, SLAB], dt.float16, tag="qT")
            out_allT = persist.tile([P, ET, SLAB], dt.float16, tag="out_allT")
            wo_sb = persist.tile([P, ET, EMB], dt.float16, tag="wo_sb")
            bv_sb = persist.tile([P, ET], dt.float32, tag="bv_sb")
            c32 = persist.tile([1, P], dt.float8e4, tag="c32")
            e_p0 = persist.tile([1, 2, SLAB], dt.float8e4, tag="e_p0")
            ones64 = persist.tile([1, HD], dt.float16, tag="ones64")

            nc.vector.memset(c32[:], RS)
            nc.vector.memset(ones64[:], 1.0)

            # projection operands, loaded on the ACT DMA queue during mask
            xt_sb = projd.tile([P, ET, N], dt.float16, tag="xt_sb")
            xtr_sb = projd.tile([P, ET, SLAB], dt.float16, tag="xtr_sb")
            wq_sb = projd.tile([P, ET, EMB], dt.float16, tag="wq_sb")
            wk_sb = projd.tile([P, ET, EMB], dt.float16, tag="wk_sb")
            wv_sb = projd.tile([P, ET, EMB], dt.float16, tag="wv_sb")
            bq_sb = projd.tile([P, ET], dt.float32, tag="bq_sb")
            bk_sb = projd.tile([P, ET], dt.float32, tag="bk_sb")

            # ---------------- phase M: mask chain (all fp8 DoubleRow) ----
            with (
                tc.tile_pool(name="dchain", bufs=1) as dchain,
                tc.tile_pool(name="btile", bufs=3) as btile,
                tc.tile_pool(name="psA", bufs=3, space="PSUM") as psA,
            ):
                D_a8 = dchain.tile([P, NB, SLAB], dt.float8e4, tag="D_a8")
                D2_8 = dchain.tile([P, NB, SLAB], dt.float8e4, tag="D2_8")
                D3_8 = dchain.tile([P, NB, SLAB], dt.float8e4, tag="D3_8")
                e_raw = dchain.tile([P, 2, SLAB], dt.float8e4, tag="e_raw")
                e_tmp = dchain.tile([P, SLAB], dt.float16, tag="e_tmp")
                corr2_d = dchain.tile([P, SLAB], dt.float16, tag="corr2_d")
                corr2_raw = dchain.tile([P, SLAB], dt.float16, tag="corr2_raw")
                corr3F = dchain.tile([P, SLAB], dt.float16, tag="corr3F")

                # startup: first B tile, then D1 in chunks (first DR pair
                # only needs blocks 0-1)
                bm8_first = btile.tile([P, NB, P], dt.float8e4, tag="bm8")
                nc.sync.dma_start(bm8_first[:], bt8[NB - 1])
                d18r = d18.rearrange("(kb p) q -> p kb q", p=P)
                for c in range(4):
                    nc.sync.dma_start(
                        D_a8[:, 8 * c : 8 * c + 8, :], d18r[:, 8 * c : 8 * c + 8, :]
                    )

                # level 1: D2 = B^T D1 (exact 0/1 operands)
                for m in M_ORDER:
                    if m == NB - 1:
                        bm8 = bm8_first
                    else:
                        bm8 = btile.tile([P, NB, P], dt.float8e4, tag="bm8")
                        nc.sync.dma_start(bm8[:], bt8[m])
                    ps = psA.tile([P, SLAB], dt.float32, tag="acc")
                    for k2 in range(NB // 2):
                        nc.tensor.matmul(
                            ps[:],
                            bm8[:, 2 * k2 : 2 * k2 + 2, :],
                            D_a8[:, 2 * k2 : 2 * k2 + 2, :],
                            start=(k2 == 0),
                            stop=(k2 == NB // 2 - 1),
                            perf_mode=mybir.MatmulPerfMode.DoubleRow,
                        )
                    nc.scalar.mul(D2_8[:, m, :], ps[:], S2)
                    if m == NB - 1:
                        # E2*2^5 = ps[virt] - 32*D2_8[virt]  (residual; only
                        # row 127 = virtual node is used, computed full-block
                        # because DVE requires partition base 0)
                        nc.vector.scalar_tensor_tensor(
                            out=e_raw[:, 0, :],
                            in0=D2_8[:, NB - 1, :],
                            scalar=-32.0,
                            in1=ps[:],
                            op0=ALU.mult,
                            op1=ALU.add,
                        )
                        nc.sync.dma_start(e_p0[0:1, 0, :], e_raw[P - 1 : P, 0, :])
                        # broadcast E2 across partitions once (folded into the
                        # level-2 psum consumers instead of per-block rank-1s)
                        ps_c = psA.tile([P, SLAB], dt.float32, tag="acc")
                        nc.tensor.matmul(
                            ps_c[:], c32[:], e_p0[0:1, 0, :], start=True, stop=True
                        )
                        nc.scalar.mul(corr2_d[:], ps_c[:], S3)
                        nc.scalar.copy(corr2_raw[:], ps_c[:])
                        # prefetch projection operands on the ACT DMA queue
                        # (separate hw queue; does not delay the bt8 stream)
                        nc.scalar.dma_start(
                            xt_sb[:], xt.rearrange("(t p) n -> p t n", p=P)
                        )
                        nc.scalar.dma_start(
                            xtr_sb[:], xtr.rearrange("(t p) q -> p t q", p=P)
                        )
                        nc.scalar.dma_start(
                            wq_sb[:], wq.rearrange("(t p) c -> p t c", p=P)
                        )
                        nc.scalar.dma_start(
                            wk_sb[:], wk.rearrange("(t p) c -> p t c", p=P)
                        )
                        nc.scalar.dma_start(
                            wv_sb[:], wv.rearrange("(t p) c -> p t c", p=P)
                        )
                        nc.scalar.dma_start(bq_sb[:], bq.rearrange("(t p) -> p t", p=P))
                        nc.scalar.dma_start(bk_sb[:], bk.rearrange("(t p) -> p t", p=P))
                        nc.scalar.dma_start(
                            wo_sb[:], wo.rearrange("(t p) c -> p t c", p=P)
                        )
                        nc.scalar.dma_start(bv_sb[:], bv.rearrange("(t p) -> p t", p=P))

                # levels 2, 3: fp8 DR; the virtual-row fp8 residual enters via
                # broadcast correction tiles added in the psum consumers.
                # F = 0.125*B^4 exactly: the virtual node's all-ones row/col
                # makes the B^2/B^3/eye max-terms redundant (B^4 >= 2*B^3 and
                # >= 4*B^2 entrywise, proven + verified).
                for level in (2, 3):
                    src = D2_8 if level == 2 else D3_8
                    for m in M_ORDER:
                        bm8 = btile.tile([P, NB, P], dt.float8e4, tag="bm8")
                        nc.sync.dma_start(bm8[:], bt8[m])
                        ps = psA.tile([P, SLAB], dt.float32, tag="acc")
                        for k2 in range(NB // 2):
                            nc.tensor.matmul(
                                ps[:],
                                bm8[:, 2 * k2 : 2 * k2 + 2, :],
                                src[:, 2 * k2 : 2 * k2 + 2, :],
                                start=(k2 == 0),
                                stop=(k2 == NB // 2 - 1),
                                perf_mode=mybir.MatmulPerfMode.DoubleRow,
                            )
                        if level == 3:
                            # F = (ps + E3_bcast) * 0.25
                            nc.vector.scalar_tensor_tensor(
                                out=F[:, m, :], in0=ps[:], scalar=C4,
                                in1=corr3F[:], op0=ALU.mult, op1=ALU.add,
                            )
                        if level == 2:
                            # D3_8 = (ps + E2_bcast) * S3
                            nc.vector.scalar_tensor_tensor(
                                out=D3_8[:, m, :], in0=ps[:], scalar=S3,
                                in1=corr2_d[:], op0=ALU.mult, op1=ALU.add,
                            )
                            if m == NB - 1:
                                nc.vector.scalar_tensor_tensor(
                                    out=e_tmp[:],
                                    in0=D3_8[:, NB - 1, :],
                                    scalar=-32.0,
                                    in1=ps[:],
                                    op0=ALU.mult,
                                    op1=ALU.add,
                                )
                                nc.vector.tensor_tensor(
                                    out=e_raw[:, 1, :], in0=e_tmp[:],
                                    in1=corr2_raw[:], op=ALU.add,
                                )
                                nc.sync.dma_start(
                                    e_p0[0:1, 1, :], e_raw[P - 1 : P, 1, :]
                                )
                                ps_c = psA.tile([P, SLAB], dt.float32, tag="acc")
                                nc.tensor.matmul(
                                    ps_c[:], c32[:], e_p0[0:1, 1, :],
                                    start=True, stop=True,
                                )
                                nc.scalar.mul(corr3F[:], ps_c[:], C4)

            # ---------------- phase P: projections ----------------
            with tc.tile_pool(name="kv", bufs=1) as kv:
                kT = kv.tile([P, ET, N], dt.float16, tag="kT")
                v_sb = kv.tile([P, NB, HEADS, HD + 1], dt.float16, tag="v_sb")

                with tc.tile_pool(name="psP", bufs=3, space="PSUM") as psP:
                    # qT[hd, q] = (Wq' X_r^T) + bq'
                    for hb in range(ET):
                        ps = psP.tile([P, SLAB], dt.float32, tag="acc")
                        for t in range(ET):
                            nc.tensor.matmul(
                                ps[:],
                                wq_sb[:, t, hb * P : (hb + 1) * P],
                                xtr_sb[:, t, :],
                                start=(t == 0),
                                stop=(t == ET - 1),
                            )
                        nc.scalar.activation(
                            qT[:, hb, :], ps[:], AF.Identity, bias=bq_sb[:, hb : hb + 1]
                        )

                    # kT[hd, key] = (Wk X^T) + bk
                    for hb in range(ET):
                        for nck in range(N // SLAB):
                            ps = psP.tile([P, SLAB], dt.float32, tag="acc")
                            for t in range(ET):
                                nc.tensor.matmul(
                                    ps[:],
                                    wk_sb[:, t, hb * P : (hb + 1) * P],
                                    xt_sb[:, t, nck * SLAB : (nck + 1) * SLAB],
                                    start=(t == 0),
                                    stop=(t == ET - 1),
                                )
                            nc.scalar.activation(
                                kT[:, hb, nck * SLAB : (nck + 1) * SLAB],
                                ps[:],
                                AF.Identity,
                                bias=bk_sb[:, hb : hb + 1],
                            )

                    # V[node, hd] = X Wv   (bv added later per-partition)
                    for nb in range(NB):
                        ps = psP.tile([P, SLAB], dt.float32, tag="acc")
                        for t in range(ET):
                            nc.tensor.matmul(
                                ps[:],
                                xt_sb[:, t, nb * P : (nb + 1) * P],
                                wv_sb[:, t, :],
                                start=(t == 0),
                                stop=(t == ET - 1),
                            )
                        nc.vector.tensor_copy(
                            v_sb[:, nb, :, 0:HD],
                            ps.rearrange("p (h d) -> p h d", h=HEADS),
                        )
                    nc.vector.memset(v_sb[:, :, :, HD : HD + 1], 1.0)

                # ---------------- phase A: attention (row-tiled head pairs,
                # deferred pair normalization) ----------------
                with (
                    tc.tile_pool(name="attn", bufs=2) as attn,
                    tc.tile_pool(name="small", bufs=1) as small,
                    tc.tile_pool(name="psQK", bufs=2, space="PSUM") as psQK,
                    tc.tile_pool(name="psAV", bufs=2, space="PSUM") as psAV,
                ):
                    def emit_normalize(po_pair, t):
                        for l in range(2):
                            po = l * HD
                            row = small.tile([1, SLAB], dt.float32, tag="row")
                            rscr = small.tile([1, SLAB], dt.float32, tag="rscr")
                            nc.vector.tensor_copy(row[:], po_pair[HD : HD + 1, l, :])
                            nc.vector.reciprocal_approx_accurate(
                                row[:], row[:], rscr[:]
                            )
                            row16 = small.tile([1, SLAB], dt.float16, tag="row16")
                            nc.vector.tensor_copy(row16[:], row[:])
                            rps = psQK.tile([P, 2, SLAB], dt.float32, tag="qk")
                            nc.tensor.matmul(
                                rps[0:HD, 0, :], ones64[:], row16[:],
                                start=True, stop=True,
                            )
                            r_sb = small.tile([HD, SLAB], dt.float32, tag="r_sb")
                            nc.scalar.copy(r_sb[:], rps[0:HD, 0, :])
                            otmp = small.tile([HD, SLAB], dt.float32, tag="otmp")
                            nc.vector.tensor_tensor(
                                out=otmp[:], in0=po_pair[0:HD, l, :],
                                in1=r_sb[:], op=ALU.mult,
                            )
                            nc.vector.tensor_scalar_add(
                                out_allT[po : po + HD, t, :], otmp[:],
                                bv_sb[po : po + HD, t : t + 1],
                            )

                    pending = None
                    for t in range(HEADS // 2):
                        po_pair = psAV.tile([P, 2, SLAB], dt.float32, tag="pout")
                        for ck in range(NB // CHUNK):
                            sexp = attn.tile(
                                [P, CHUNK, 2, SLAB], dt.float16, tag="sexp"
                            )
                            for kc in range(CHUNK):
                                kb = ck * CHUNK + kc
                                ps = psQK.tile([P, 2, SLAB], dt.float32, tag="qk")
                                nc.tensor.matmul(
                                    ps[:, 0, :],
                                    kT[0:HD, t, kb * P : (kb + 1) * P],
                                    qT[0:HD, t, :],
                                    start=True,
                                    stop=True,
                                    tile_position=(0, 0),
                                )
                                nc.tensor.matmul(
                                    ps[:, 1, :],
                                    kT[HD:P, t, kb * P : (kb + 1) * P],
                                    qT[HD:P, t, :],
                                    start=True,
                                    stop=True,
                                    tile_position=(HD, 0),
                                )
                                nc.scalar.activation(sexp[:, kc, :, :], ps[:], AF.Exp)
                                nc.vector.tensor_tensor(
                                    out=sexp[:, kc, 0, :], in0=sexp[:, kc, 0, :],
                                    in1=F[:, kb, :], op=ALU.mult,
                                )
                                nc.vector.tensor_tensor(
                                    out=sexp[:, kc, 1, :], in0=sexp[:, kc, 1, :],
                                    in1=F[:, kb, :], op=ALU.mult,
                                )
                            if ck == 0 and pending is not None:
                                # previous pair's normalization lands behind
                                # this pair's first QK chunk so the PE never
                                # waits on the DVE reciprocal chain
                                emit_normalize(*pending)
                                pending = None
                            for kc in range(CHUNK):
                                kb = ck * CHUNK + kc
                                for l in range(2):
                                    nc.tensor.matmul(
                                        po_pair[0 : HD + 1, l, :],
                                        v_sb[:, kb, 2 * t + l, :],
                                        sexp[:, kc, l, :],
                                        start=(kb == 0),
                                        stop=(kb == NB - 1),
                                    )
                        pending = (po_pair, t)
                    emit_normalize(*pending)

                # ---------------- phase O: output projection ----------------
                with (
                    tc.tile_pool(name="osmall", bufs=2) as osmall,
                    tc.tile_pool(name="psO", bufs=2, space="PSUM") as psO,
                ):
                    for qb in range(ET):
                        ps = psO.tile([P, SLAB], dt.float32, tag="acc")
                        for t in range(ET):
                            nc.tensor.matmul(
                                ps[:],
                                out_allT[:, t, qb * P : (qb + 1) * P],
                                wo_sb[:, t, :],
                                start=(t == 0),
                                stop=(t == ET - 1),
                            )
                        fin = osmall.tile([P, SLAB], dt.float32, tag="fin")
                        nc.vector.tensor_copy(fin[:], ps[:])
                        nc.sync.dma_start(out[qb * P : (qb + 1) * P, :], fin[:])

    nc.compile()
    return nc


def _prep_host(input_embeddings, edge_index, num_nodes, Wq, bq, Wk, bk, Wv, bv, Wo, bo):
    n = int(num_nodes) + 1
    assert n == N

    B = np.zeros((n, n), dtype=np.float32)
    idx = np.arange(n)
    B[idx, idx] = 1.0
    e0 = np.asarray(edge_index[0], dtype=np.int64)
    e1 = np.asarray(edge_index[1], dtype=np.int64)
    B[e0, e1] = 1.0
    B[: n - 1, n - 1] = 1.0
    B[n - 1, : n - 1] = 1.0

    fp8 = mybir.dt.np(dt.float8e4)
    # bt8[m, p, kb, f] = B[kb*128+p, m*128+f]
    bt8 = np.ascontiguousarray(
        B.reshape(NB, P, NB, P).transpose(2, 1, 0, 3)
    ).astype(fp8)

    X = np.asarray(input_embeddings, dtype=np.float32)
    xt = np.ascontiguousarray(X.T.astype(np.float16))

    wq_h = np.ascontiguousarray((np.asarray(Wq, np.float32) * 0.125).astype(np.float16))
    wk_h = np.ascontiguousarray(np.asarray(Wk, np.float32).astype(np.float16))
    wv_h = np.ascontiguousarray(np.asarray(Wv, np.float32).astype(np.float16))
    wo_h = np.ascontiguousarray(np.asarray(Wo, np.float32).astype(np.float16))
    bq_h = np.ascontiguousarray(np.asarray(bq, np.float32) * 0.125)
    bk_h = np.ascontiguousarray(np.asarray(bk, np.float32))
    bv_h = np.ascontiguousarray(np.asarray(bv, np.float32))

    in_maps = []
    for core in range(NCORES):
        r0 = core * SLAB
        d18_a = np.ascontiguousarray(B[r0 : r0 + SLAB, :].T).astype(fp8)
        xtr = np.ascontiguousarray(xt[:, r0 : r0 + SLAB])
        in_maps.append(
            {
                "bt8": bt8,
                "d18": d18_a,
                "xt": xt,
                "xtr": xtr,
                "wq": wq_h,
                "wk": wk_h,
                "wv": wv_h,
                "wo": wo_h,
                "bq": bq_h,
                "bk": bk_h,
                "bv": bv_h,
            }
        )
    return in_maps


def kernel(**inputs) -> np.ndarray:
    if "nc" not in _NC_CACHE:
        _NC_CACHE["nc"] = build_bass()
    nc = _NC_CACHE["nc"]

    in_maps = _prep_host(**inputs)
    res = run_bass_kernel_spmd(nc, in_maps, core_ids=list(range(NCORES)))
    global LAST_RESULT
    LAST_RESULT = res
    bo = np.asarray(inputs["bo"], dtype=np.float32)
    slabs = [res.results[c]["out"] for c in range(NCORES)]
    return (np.concatenate(slabs, axis=0) + bo[None, :]).astype(np.float32)


if __name__ == "__main__":
    import reference

    inputs = {k: np.asarray(v) if not np.isscalar(v) else v for k, v in reference.setup_inputs().items()}
    got = kernel(**inputs)
    print("kernel output:", got.shape, got.dtype)
